# revision 54
# baseline (speedup 1.0000x reference)
"""Trainium2 Bass kernel for nn_Attention_197568495719.

Full attention layer: QKV projection + RoPE + int8 KV quant-dequant + GQA
causal SDPA + output projection.  B=2, S=2048, D=2048, 16 q heads / 4 kv
heads, head_dim=128.

Sharding: 8 cores = 2 (batch) x 4 (kv-head groups).  Core (b, g) computes
batch b with q heads 4g..4g+3 and kv head g (tensor parallel on heads:
wq/wk/wv split on output dim, wo on input dim).  Each core produces a
partial outT = (attn @ wo_g).T in [D, S] layout; the host sums the 4
group partials per batch and transposes back.

Design (v3, ~287-304us measured depending on the device's bimodal
clock mode; v2 was ~288-342us, v1 ~485us):
- Everything on the PE is bf16 (hardware fp32r "HIGH" mode multiplies
  with bf16-truncated operands anyway, but pays a ~70ns un-hidden
  fp32 LDWEIGHTS per matmul since FWL is fp32-disabled -- bf16 is
  numerically equivalent and strictly faster).  Host pre-arranges
  dataT/wq/wkv partition-major so every DMA element is >=4KB (512B
  elements run ~3x slower, and small-element descriptors starve their
  whole queue at the packet-round-robin arbiter -- tiny consts ride at
  queue tails).  Initial loads fan out over all three DMA-capable
  queues (sync/gpsimd/scalar) ordered by first use; wo loads ride the
  idle mid-phase-1 DMA window.  A bf16 scratch-matmul burst (512- then
  128-col) bridges the DMA-bound head so the HAM clock-gate never
  re-throttles before real work arrives.
- Phase 1 (projections): 512-token chunks; k/v projected directly into
  [token, dim] tiles so the int8 quant path needs no PE transposes in;
  k RoPE runs along the free axis with a sign-folded sin table; q RoPE
  in place per (head, chunk) with rot matmuls deferred behind all four
  head projections.  Quant rounding uses the fp32 +-1.5*2^23 magic-add
  (exact round-half-to-even, matching jnp.round).
- Phase 2 (attention): ki tiles processed in units of 2 with
  [128,2,QC] wide tiles spanning 2 PSUM banks: one ACT exp per unit
  (amortizing the 352-cycle ACT pipeline fill; ACT is the co-critical
  engine -- diagonal units score 128 extra masked-never-read columns
  on their second half so the whole unit exps in one instruction), one
  wide DVE staging copy per pair, wide finalize ops.  Scores race
  LAG=2 units ahead of the accumulating matmuls.  Causal masking
  multiplies only the 128x128 triangular block per diagonal tile on
  GPSIMD; the accumulating matmuls are trimmed to the exact live
  q-range (128j).  The softmax denominator rides the
  PE as ones-matmuls; off-diagonal units' two exp tiles are pre-summed
  element-wise off the PE (chain-free, alternating GPSIMD/DVE by
  stream) so one ones-matmul covers both ki tiles.  (Fully chained
  engine-side accumulation and DVE reciprocal were both tried and
  measured slower: the chains starve the PE, and DVE reciprocal costs
  ~4.3us per [128,2,512].)  1/Z = exp(-ln(Z)) on ACT: Ln/Exp share an
  ACT function table so no ACT_TABLE_LOAD splits the exp stream; each
  pair's finalize is deferred into the next pair (Ln at pair start
  frees the PSUM slot for reuse).  Each chunk's output projection,
  with each head's q-rope staggered one projection behind its PSUM
  copy and the kq transposes after all projections (clear of the DVE
  quant chain), runs as a dense
  software-pipelined PE block at the next chunk boundary -- all 8 PSUM
  banks are free there, the ACT engine gets a breather between
  exp-heavy pairs, and the first block unit rides the attn slot so it
  never waits on the exp backlog.  outT partials are stored bf16 in dt
  pairs (one wide cast alternating DVE/ACT, one store alternating
  sync/gpsimd); the host accumulates the 4 head-group partials in
  fp32.
"""

import numpy as np

import bass_rust
import concourse.bass as bass
import concourse.tile as tile
import concourse.mybir as mybir
from concourse.bass_utils import run_bass_kernel_spmd

B, S, D = 2, 2048, 2048
NH, NKV, HD = 16, 4, 128
GQ = 512            # q dims per core (4 heads)
NKO = D // 128      # 16 contraction tiles
PC = 512            # projection/attention chunk width (tokens)
NPC = S // PC       # 4
QC = 512
NQC = S // QC       # 4
MAGIC = float(np.float32(12582912.0))  # 1.5 * 2**23
SM_SCALE = 1.0 / float(np.sqrt(HD))

F32 = mybir.dt.float32
F32R = mybir.dt.float32r
BF16 = mybir.dt.bfloat16
MULT = mybir.AluOpType.mult
ADD = mybir.AluOpType.add
EXP = mybir.ActivationFunctionType.Exp

_CACHE = {}

# retained after each kernel() call so test harnesses can read profiling info
LAST_RESULTS = None


def _split_multi_waits(nc):
    """This walrus build caps sync waits at 1 per instruction.  Hoist extra
    waits onto single-wait NoOps immediately preceding the instruction on
    the same engine (identical semantics: the engine is in-order)."""
    for f in nc.m.functions:
        for bb in f.blocks:
            new = []
            for inst in bb.instructions:
                si = inst.sync_info
                if si is None:
                    new.append(inst)
                    continue
                waits = list(si.on_wait)
                if len(waits) > 1:
                    for k, w in enumerate(waits[:-1]):
                        nop = mybir.InstNoOp(name=f"{inst.name}-w{k}", ins=[], outs=[])
                        nop.engine = inst.engine
                        nop.sync_info = bass_rust.SyncInfo(on_wait=[w], on_update=[])
                        new.append(nop)
                    inst.sync_info = bass_rust.SyncInfo(
                        on_wait=[waits[-1]], on_update=list(si.on_update)
                    )
                new.append(inst)
            bb.instructions = new


def _host_consts():
    theta = 10000.0
    angles = 1.0 / theta ** (np.arange(0, HD, 2, dtype=np.float32) / HD)
    emb = np.outer(np.arange(S, dtype=np.float32), angles)
    emb = np.concatenate([emb, emb], axis=-1)          # [S, HD]
    cos = np.cos(emb).astype(np.float32)               # [S, HD]
    sin = np.sin(emb).astype(np.float32)
    cosT = np.ascontiguousarray(cos.T)                 # [128, S]
    sinT = np.ascontiguousarray(sin.T)

    # [t, d]-layout tables for k rope: [p, t_tile, hd]
    ctd = np.ascontiguousarray(cos.reshape(S // 128, 128, HD).transpose(1, 0, 2))
    std = sin.reshape(S // 128, 128, HD).transpose(1, 0, 2).copy()
    sgn = std.copy()
    sgn[:, :, : HD // 2] = -std[:, :, : HD // 2]       # sign-folded sin
    sgn = np.ascontiguousarray(sgn)

    rot = np.zeros((128, 128), dtype=np.float32)       # lhsT of rotate_half
    for i in range(64):
        rot[i, i + 64] = 1.0
        rot[i + 64, i] = -1.0

    p = np.arange(128)[:, None]
    f = np.arange(128)[None, :]
    tril = (p <= f).astype(np.float32)                 # key p visible to q f

    ones = np.ones((128, 128), dtype=np.float32)
    ident = np.eye(128, dtype=np.float32)
    import ml_dtypes
    bf16 = ml_dtypes.bfloat16
    return {
        "cosT": cosT.astype(bf16), "sinT": sinT.astype(bf16),
        "ctd": ctd.astype(bf16), "sgn": sgn.astype(bf16),
        "rot": rot.astype(bf16), "tril": tril.astype(bf16),
        "ones": ones.astype(bf16), "ident": ident,
    }


def _build_nc():
    nc = bass.Bass("TRN2", target_bir_lowering=False, debug=False)

    # host pre-arranges dataT/wq/wkv into partition-major layouts so every
    # DMA element is >=4KB contiguous (512B elements run ~3x slower)
    dataT = nc.dram_tensor("dataT", [128, NPC, NKO, PC], BF16,
                           kind="ExternalInput").ap()
    wq = nc.dram_tensor("wq", [128, NKO, GQ], BF16, kind="ExternalInput").ap()
    wkv = nc.dram_tensor("wkv", [128, NKO, 2 * HD], BF16,
                         kind="ExternalInput").ap()
    wo = nc.dram_tensor("wo", [GQ, D], BF16, kind="ExternalInput").ap()
    cosT_d = nc.dram_tensor("cosT", [128, S], BF16, kind="ExternalInput").ap()
    sinT_d = nc.dram_tensor("sinT", [128, S], BF16, kind="ExternalInput").ap()
    ctd_d = nc.dram_tensor("ctd", [128, NKO, HD], BF16, kind="ExternalInput").ap()
    sgn_d = nc.dram_tensor("sgn", [128, NKO, HD], BF16, kind="ExternalInput").ap()
    rot_d = nc.dram_tensor("rot", [128, 128], BF16, kind="ExternalInput").ap()
    tril_d = nc.dram_tensor("tril", [128, 128], BF16, kind="ExternalInput").ap()
    ones_d = nc.dram_tensor("ones", [128, 128], BF16, kind="ExternalInput").ap()
    ident_d = nc.dram_tensor("ident", [128, 128], F32R, kind="ExternalInput").ap()
    outT = nc.dram_tensor("outT", [D, S], BF16, kind="ExternalOutput").ap()

    dataT_r = dataT                                          # [128, 4, 16, PC]
    wq_r = wq                                                # [128, 16, 512]
    wkv_r = wkv                                              # [128, 16, 256]
    wo_r = wo.rearrange("(h p) n -> p h n", p=128)           # [128, 4, S]
    outT_p = outT.rearrange("(dt p) t -> p dt t", p=128)     # [128, 16, S]

    from contextlib import ExitStack
    with tile.TileContext(nc) as tc, ExitStack() as stack:
        small_consts = stack.enter_context(tc.tile_pool(name="sconsts", bufs=1))
        rot_sb = small_consts.tile([128, 128], BF16)
        ones_sb = small_consts.tile([128, 128], BF16)
        id_sb = small_consts.tile([128, 128], F32R)
        tril_sb = small_consts.tile([128, 128], BF16)

        persist = stack.enter_context(tc.tile_pool(name="persist", bufs=1))
        xq4 = persist.tile([128, 4, S], BF16, name="xq4")    # roped q, [d, h, t]
        kt4 = persist.tile([128, 4, QC], BF16, name="kt4")   # quant k, [d, g, t]
        v_g = [persist.tile([128, 4, HD], BF16, tag=f"vg{g}", name=f"v_g{g}")
               for g in range(4)]                            # quant v, [t, j, d]
        wo_t = [persist.tile([128, S], BF16, tag=f"wo{h}", name=f"wo{h}")
                for h in range(4)]                           # loaded mid-phase-1

        GRP = 4

        # ---------------- Phase 1: projections + rope + quant ----------------
        with tc.tile_pool(name="p1consts", bufs=1) as p1c, \
             tc.tile_pool(name="wpool", bufs=1) as wpool, \
             tc.tile_pool(name="datapool", bufs=2) as datapool, \
             tc.tile_pool(name="kvstage", bufs=2) as kvstage, \
             tc.tile_pool(name="qtmp", bufs=2) as qtmp, \
             tc.tile_pool(name="t2pool", bufs=3) as t2pool, \
             tc.tile_pool(name="proj_ps", bufs=3, space="PSUM") as proj_ps, \
             tc.tile_pool(name="kv_ps", bufs=2, space="PSUM") as kv_ps, \
             tc.tile_pool(name="rope_ps", bufs=2, space="PSUM") as rope_ps, \
             tc.tile_pool(name="tp_ps", bufs=1, space="PSUM") as tp_ps:
            cos_sb = p1c.tile([128, S], BF16)
            sin_sb = p1c.tile([128, S], BF16)
            ctd_sb = p1c.tile([128, NKO, HD], BF16)
            sgn_sb = p1c.tile([128, NKO, HD], BF16)
            wq_sb = wpool.tile([128, NKO, GQ], BF16)
            wkv_sb = wpool.tile([128, NKO, 2 * HD], BF16)

            dT = {}
            for c in range(2):
                dT[c] = datapool.tile([128, NKO, PC], BF16, tag="dT",
                                      name=f"dT{c}")

            # initial loads across all 3 DMA-capable queues (sync/gpsimd/
            # scalar).  The DMA arbiter round-robins PACKETS across queues,
            # so a queue carrying small-element descriptors gets starved:
            # big 4-8KB-element transfers go first in each queue's FIFO.
            # The first-needed tensors are QUARTERED so the kv projection's
            # ko loop starts on the first 0.5MB (per-region tile deps) and
            # trickles, instead of waiting for whole halves; ctd/cos only
            # feed DVE chains with slack, so dT1 outranks them.
            nc.sync.dma_start(dT[0][:, 0:4], dataT_r[:, 0, 0:4])
            nc.gpsimd.dma_start(dT[0][:, 8:12], dataT_r[:, 0, 8:12])
            nc.scalar.dma_start(wkv_sb[:, 0:4], wkv_r[:, 0:4])
            nc.sync.dma_start(dT[0][:, 4:8], dataT_r[:, 0, 4:8])
            nc.gpsimd.dma_start(dT[0][:, 12:16], dataT_r[:, 0, 12:16])
            nc.scalar.dma_start(wkv_sb[:, 4:8], wkv_r[:, 4:8])
            nc.scalar.dma_start(wkv_sb[:, 8:12], wkv_r[:, 8:12])
            nc.scalar.dma_start(wkv_sb[:, 12:16], wkv_r[:, 12:16])
            nc.sync.dma_start(wq_sb[:, 0:4], wq_r[:, 0:4])
            nc.sync.dma_start(wq_sb[:, 4:8], wq_r[:, 4:8])
            nc.scalar.dma_start(wq_sb[:, 8:12], wq_r[:, 8:12])
            nc.scalar.dma_start(wq_sb[:, 12:16], wq_r[:, 12:16])
            nc.sync.dma_start(rot_sb[:], rot_d[:])
            nc.sync.dma_start(id_sb[:], ident_d[:])
            nc.gpsimd.dma_start(dT[1][:, 8:16], dataT_r[:, 1, 8:16])
            nc.sync.dma_start(dT[1][:, 0:8], dataT_r[:, 1, 0:8])
            nc.gpsimd.dma_start(ctd_sb[:], ctd_d[:])
            nc.gpsimd.dma_start(cos_sb[:], cosT_d[:])
            nc.scalar.dma_start(sgn_sb[:], sgn_d[:])
            nc.scalar.dma_start(sin_sb[:], sinT_d[:])
            nc.sync.dma_start(ones_sb[:], ones_d[:])
            nc.sync.dma_start(tril_sb[:], tril_d[:])

            # PE warm-up/filler: scratch matmuls during the initial DMA wait
            # keep the HAM clock-gate at K=8/8 so real work runs at full
            # clock, and bridge to the kv projection's first data (~17us in)
            # so no >3.4us idle window re-throttles the clock.
            warm = wpool.tile([128, QC], BF16, name="warm_scratch")
            nc.vector.memset(warm[:], 0.0)

            def warm_fill(n, w=QC):
                for _ in range(n):
                    wps = rope_ps.tile([128, QC], F32, tag="pr")
                    nc.tensor.matmul(wps[:, 0:w], warm[:, 0:128], warm[:, 0:w],
                                     start=True, stop=True)

            # bridge the whole DMA-bound head (~20us to the first 3MB) with
            # warms: starting the kv projection early just makes it TRICKLE
            # behind the quarter arrivals, and the repeated micro-idles pin
            # the HAM clock-gate at half clock for ~17us -- a dense late
            # start at full clock is strictly faster.  The narrow tail keeps
            # the overshoot past data-arrival under ~60ns per warm.
            warm_fill(22)
            warm_fill(60, w=128)

            def quant_group(src_ap, dst_ap):
                amax = qtmp.tile([128, GRP, 1], F32, tag="amax")
                scl = qtmp.tile([128, GRP, 1], F32, tag="scl")
                inv = qtmp.tile([128, GRP, 1], F32, tag="inv")
                xs = qtmp.tile([128, GRP, HD], F32, tag="xs")
                nc.vector.tensor_reduce(amax[:], src_ap, mybir.AxisListType.X,
                                        mybir.AluOpType.max,
                                        apply_absolute_value=True)
                nc.vector.tensor_scalar_max(amax[:], amax[:], 1e-8)
                nc.vector.tensor_scalar_mul(scl[:], amax[:], 1.0 / 127.0)
                nc.vector.reciprocal(inv[:], scl[:])
                sclb = scl[:].to_broadcast((128, GRP, HD))
                invb = inv[:].to_broadcast((128, GRP, HD))
                nc.vector.tensor_tensor(xs[:], src_ap, invb, MULT)
                nc.vector.tensor_scalar_add(xs[:], xs[:], MAGIC)
                nc.vector.tensor_scalar_add(xs[:], xs[:], -MAGIC)
                nc.vector.tensor_tensor(dst_ap, xs[:], sclb, MULT)

            for c in range(NPC):
                csl = bass.ts(c, PC)
                if c + 2 < NPC:
                    cb = c + 2
                    t_ = datapool.tile([128, NKO, PC], BF16, tag="dT",
                                       name=f"dT{cb}")
                    dT[cb] = t_
                    eng = nc.gpsimd if cb % 2 else nc.sync
                    eng.dma_start(t_[:, 0:8], dataT_r[:, cb, 0:8])
                    eng.dma_start(t_[:, 8:16], dataT_r[:, cb, 8:16])
                if c == 2:
                    # wo is first needed by the out-projection block at the
                    # first phase-2 chunk boundary; load it mid-phase-1
                    # while the DMA queues are otherwise idle
                    for h in range(4):
                        eng = nc.sync if h % 2 else nc.scalar
                        eng.dma_start(wo_t[h][:], wo_r[:, h])

                # --- k/v projection straight into [t, d] tiles ---
                kv_td = kvstage.tile([128, GRP, 2 * HD], F32, tag="kvtd",
                                     name=f"kvtd{c}")
                for j in range(GRP):
                    pkv = kv_ps.tile([128, 2 * HD], F32, tag="pkv")
                    for ko in range(NKO):
                        nc.tensor.matmul(pkv[:],
                                         dT[c][:, ko, bass.ds(j * 128, 128)],
                                         wkv_sb[:, ko],
                                         start=(ko == 0), stop=(ko == NKO - 1))
                    nc.scalar.copy(kv_td[:, j, :], pkv[:])

                # --- k rope along free axis (sign-folded sin table) ---
                kr = kvstage.tile([128, GRP, HD], F32, tag="kr", name=f"kr{c}")
                t2k = qtmp.tile([128, GRP, HD], F32, tag="t2k")
                tsl = bass.ts(c, GRP)  # 4 token tiles of this group
                nc.vector.tensor_tensor(kr[:], kv_td[:, :, 0:HD],
                                        ctd_sb[:, tsl], MULT)
                nc.vector.tensor_tensor(t2k[:, :, 0:64],
                                        kv_td[:, :, 64:HD],
                                        sgn_sb[:, tsl, 0:64], MULT)
                nc.vector.tensor_tensor(t2k[:, :, 64:HD],
                                        kv_td[:, :, 0:64],
                                        sgn_sb[:, tsl, 64:HD], MULT)
                nc.vector.tensor_tensor(kr[:], kr[:], t2k[:], ADD)

                # --- int8 quant-dequant (k roped, v raw); v rides here too
                # so the chunk's DVE work finishes early: the last chunk's
                # DVE tail otherwise delays the phase-2 pool handover ---
                kq = kvstage.tile([128, GRP, HD], F32R, tag="kq", name=f"kq{c}")
                quant_group(kr[:], kq[:])
                quant_group(kv_td[:, :, HD:], v_g[c][:])

                # --- q projection per head, with each head's rope staggered
                # one projection behind its copy (so the PE never waits on
                # the ACT copy), and the kq transposes last (the DVE quant
                # chain is guaranteed done by then, and kt4 isn't read until
                # phase 2) ---
                def emit_qproj(h):
                    pq = proj_ps.tile([128, QC], F32, tag="pq",
                                      name=f"pq{c}_{h}")
                    for ko in range(NKO):
                        nc.tensor.matmul(pq[:], wq_sb[:, ko, bass.ts(h, 128)],
                                         dT[c][:, ko],
                                         start=(ko == 0), stop=(ko == NKO - 1))
                    nc.scalar.copy(xq4[:, h, csl], pq[:])

                def emit_rope(h):
                    pr = rope_ps.tile([128, QC], F32, tag="pr")
                    nc.tensor.matmul(pr[:], rot_sb[:], xq4[:, h, csl],
                                     start=True, stop=True)
                    t1 = t2pool.tile([128, QC], BF16, tag="t1")
                    t2 = t2pool.tile([128, QC], BF16, tag="t2")
                    nc.vector.tensor_tensor(t1[:], xq4[:, h, csl],
                                            cos_sb[:, csl], MULT)
                    nc.vector.tensor_tensor(t2[:], pr[:], sin_sb[:, csl], MULT)
                    nc.vector.tensor_tensor(xq4[:, h, csl], t1[:], t2[:], ADD)

                emit_qproj(0)
                emit_qproj(1)
                emit_rope(0)
                emit_qproj(2)
                emit_rope(1)
                emit_qproj(3)
                emit_rope(2)
                for j in range(GRP):
                    pt = tp_ps.tile([128, 128], F32R, tag="tp")
                    nc.tensor.transpose(pt[:], kq[:, j, :], id_sb[:])
                    nc.scalar.copy(kt4[:, c, bass.ts(j, 128)], pt[:])
                emit_rope(3)

        # ---------------- Phase 2: attention + output projection ----------------
        # ki tiles are processed in units of 2 with [128,2,QC] "wide" tiles
        # spanning 2 PSUM banks / 2KB-per-partition SBUF spans: one exp per
        # off-diagonal unit (amortizes the 352-cycle ACT pipeline fill), one
        # staging copy / store per unit.  The softmax denominator rides the
        # PE as per-ki ones-matmuls (engine-side accumulation measured ~2x
        # slower and starves the PE with serial chains).  Each chunk's
        # output projection runs as a dense PE block at the next chunk
        # boundary, when all 8 PSUM banks are free and the ACT engine gets
        # a breather between exp-heavy pairs.
        with tc.tile_pool(name="attn_sb", bufs=5) as attn_sb, \
             tc.tile_pool(name="exp_pool", bufs=7) as exp_pool, \
             tc.tile_pool(name="araw", bufs=3) as araw_pool, \
             tc.tile_pool(name="rc4p", bufs=2) as rc_pool, \
             tc.tile_pool(name="psum_sb", bufs=5) as psum_pool, \
             tc.tile_pool(name="outstage", bufs=4) as outstage, \
             tc.tile_pool(name="score_ps", bufs=2, space="PSUM") as score_ps, \
             tc.tile_pool(name="attn_ps", bufs=1, space="PSUM") as attn_ps, \
             tc.tile_pool(name="pss_ps", bufs=1, space="PSUM") as pss_ps:

            def out_proj_block(c_prev, tiles, fin):
                # chunk-boundary block: run the previous pair's softmax
                # finalize, then the whole [D, QC] output projection of
                # chunk c_prev software-pipelined 3 units deep over the 4
                # wide PSUM slots (all free at a chunk boundary); the h0/h1
                # lead covers the finalize chain before h2/h3 need its
                # at-tiles.  pu0 rides the attn slot (freed by the DVE ar2
                # copy) so the block's first matmuls never wait on the last
                # pair's ACT exp backlog that still holds the score slots
                fin()
                pools = [(attn_ps, "pa2"), (score_ps, "ps2"),
                         (score_ps, "ps2"), (pss_ps, "pss2")]
                pos = {}

                def finishp(pu):
                    po2 = pos.pop(pu)
                    for half in range(2):
                        dt_ = 2 * pu + half
                        for h2 in (2, 3):
                            at2, sti = tiles[h2]
                            nc.tensor.matmul(po2[:, half],
                                             wo_t[h2][:, bass.ts(dt_, 128)],
                                             at2[:, sti],
                                             start=False, stop=(h2 == 3))
                    ot2 = outstage.tile([128, 2, QC], BF16, tag="ot")
                    if pu % 2:
                        nc.vector.tensor_copy(ot2[:], po2[:])
                    else:
                        nc.scalar.copy(ot2[:], po2[:])
                    eng = nc.gpsimd if pu % 2 else nc.sync
                    eng.dma_start(outT_p[:, 2 * pu:2 * pu + 2,
                                         bass.ts(c_prev, QC)], ot2[:])

                for pu in range(NKO // 2):
                    pool, tag = pools[pu % 4]
                    po2 = pool.tile([128, 2, QC], F32, tag=tag)
                    pos[pu] = po2
                    for half in range(2):
                        dt_ = 2 * pu + half
                        for h2 in (0, 1):
                            at2, sti = tiles[h2]
                            nc.tensor.matmul(po2[:, half],
                                             wo_t[h2][:, bass.ts(dt_, 128)],
                                             at2[:, sti],
                                             start=(h2 == 0), stop=False)
                    if pu >= 3:
                        finishp(pu - 3)
                for pu in range(NKO // 2 - 3, NKO // 2):
                    finishp(pu)

            LAG = 2  # units the score/exp pipeline leads the pa matmuls by

            def emit_pair(c, hA, hB, attn_tiles, carry_in):
                nki = 4 * (c + 1)
                U = nki // 2
                streams = (hA, hB)
                if carry_in is not None:
                    # previous pair's Ln runs first so its pss slot frees
                    # before this pair's ones-matmuls need it
                    carry_in[0]()
                pa2 = attn_ps.tile([128, 2, QC], F32, tag="pa2",
                                   name=f"pa2_{c}_{hA}")
                pss2 = pss_ps.tile([128, 2, QC], F32, tag="pss2",
                                   name=f"pss2_{c}_{hA}")

                def emit_acc(u, et2s, qo, psms):
                    for st in range(2):
                        if psms[st] is not None:
                            # off-diagonal unit: its two exp tiles were
                            # pre-summed element-wise off the PE, so one
                            # ones-matmul covers both ki tiles
                            nc.tensor.matmul(
                                pss2[:, st], ones_sb[:], psms[st][:],
                                start=(2 * u == 0),
                                stop=(2 * u + 1 == nki - 1))
                        for half in range(2):
                            ki = 2 * u + half
                            q = qo[half]
                            if psms[st] is None:
                                nc.tensor.matmul(
                                    pss2[:, st, q:], ones_sb[:],
                                    et2s[st][:, half, q:],
                                    start=(ki == 0), stop=(ki == nki - 1))
                            nc.tensor.matmul(
                                pa2[:, st, q:], v_g[ki // 4][:, ki % 4],
                                et2s[st][:, half, q:],
                                start=(ki == 0), stop=(ki == nki - 1))

                pending = []
                for u in range(U):
                    k0 = 2 * u
                    diag = k0 >= 4 * c
                    qo = (128 * (k0 - 4 * c), 128 * (k0 + 1 - 4 * c)) \
                        if diag else (0, 0)
                    et2s = []
                    psms = []
                    q0 = qo[0]
                    for st in range(2):
                        h = streams[st]
                        ps2 = score_ps.tile([128, 2, QC], F32, tag="ps2")
                        for half in range(2):
                            # both halves score from q0: the diagonal
                            # half-1 computes 128 extra (masked, never
                            # read) columns so the unit exps as ONE wide
                            # ACT instruction -- ACT is the co-critical
                            # engine, the extra PE columns are cheap
                            nc.tensor.matmul(
                                ps2[:, half, q0:],
                                kt4[:, (k0 + half) // 4,
                                    bass.ts((k0 + half) % 4, 128)],
                                xq4[:, h, bass.ds(c * QC + q0, QC - q0)],
                                start=True, stop=True)
                        et2 = exp_pool.tile([128, 2, QC], BF16, tag="et2")
                        et2s.append(et2)
                        nc.scalar.activation(et2[:, :, q0:], ps2[:, :, q0:],
                                             EXP, scale=SM_SCALE)
                        if diag:
                            for half in range(2):
                                q = qo[half]
                                nc.gpsimd.tensor_tensor(
                                    et2[:, half, q:q + 128],
                                    et2[:, half, q:q + 128],
                                    tril_sb[:], MULT)
                            psms.append(None)
                        else:
                            # chain-free pairwise sum of the unit's two exp
                            # tiles (alternating engines by stream) halves
                            # the denominator's PE ones-matmul columns; it
                            # has LAG units of slack before emit_acc reads it
                            psm = psum_pool.tile([128, QC], BF16, tag="psm")
                            peng = nc.gpsimd if st == 0 else nc.vector
                            peng.tensor_tensor(psm[:], et2[:, 0], et2[:, 1],
                                               ADD)
                            psms.append(psm)
                    pending.append((u, et2s, qo, psms))
                    if u >= LAG:
                        emit_acc(*pending.pop(0))
                    if u == 1 and carry_in is not None:
                        carry_in[1]()
                for item in pending:
                    emit_acc(*item)
                # stage the attention accumulator out of PSUM (one wide copy)
                ar2 = araw_pool.tile([128, 2, QC], F32, tag="araw",
                                     name=f"ar2_{c}_{hA}")
                nc.vector.tensor_copy(ar2[:], pa2[:])

                # 1/Z = exp(-ln(Z)) on ACT: Ln and Exp share an ACT function
                # table, so no ACT_TABLE_LOAD ever splits the exp stream,
                # and at ~2.4us the pair is far cheaper than a DVE
                # reciprocal (~4.3us for [128,2,512] -- measured).  fin_a
                # (Ln, reading the PSUM accumulator directly) runs at the
                # next pair's start; fin_b at its second unit.
                state = {}

                def fin_a():
                    lnt = rc_pool.tile([128, 2, QC], F32, tag="lnt")
                    state["lnt"] = lnt
                    nc.scalar.activation(lnt[:], pss2[:],
                                         mybir.ActivationFunctionType.Ln)

                def fin_b():
                    rc2 = rc_pool.tile([128, 2, QC], F32, tag="rc4")
                    nc.scalar.activation(rc2[:], state["lnt"][:], EXP,
                                         scale=-1.0)
                    at2 = attn_sb.tile([128, 2, QC], BF16, tag="attnT")
                    # per-stream multiplies: stream 0's at-tile lands ~0.6us
                    # earlier, unblocking the out-proj block's h2 matmuls
                    for st in range(2):
                        nc.vector.tensor_tensor(at2[:, st], ar2[:, st],
                                                rc2[:, st], MULT)
                        attn_tiles[streams[st]] = (at2, st)

                def fin_tail():
                    fin_a()
                    fin_b()
                return fin_a, fin_b, fin_tail

            prev = None
            for c in range(NQC):
                attn_tiles = {}
                if prev is not None:
                    out_proj_block(prev[0], prev[1], prev[2])
                carry = emit_pair(c, 0, 1, attn_tiles, None)
                carry = emit_pair(c, 2, 3, attn_tiles, carry)
                prev = (c, attn_tiles, carry[2])
            out_proj_block(prev[0], prev[1], prev[2])

    _split_multi_waits(nc)
    return nc


def _get_state():
    if "nc" not in _CACHE:
        _CACHE["nc"] = _build_nc()
        _CACHE["consts"] = _host_consts()
    return _CACHE["nc"], _CACHE["consts"]


def kernel(data=None, mask=None, wq=None, wk=None, wv=None, wo=None, **extra):
    global LAST_RESULTS
    import ml_dtypes
    bf16 = ml_dtypes.bfloat16
    nc, consts = _get_state()

    data = np.asarray(data, dtype=np.float32)
    wq = np.asarray(wq, dtype=np.float32)
    wk = np.asarray(wk, dtype=np.float32)
    wv = np.asarray(wv, dtype=np.float32)
    wo = np.asarray(wo, dtype=np.float32)

    in_maps = []
    # dataT host layout [128, chunk, ko, t]: every DMA element is >=1KB and
    # per-(partition, chunk) spans are 16KB contiguous
    dTs = [np.ascontiguousarray(
        data[b].T.reshape(NKO, 128, NPC, PC).transpose(1, 2, 0, 3)
    ).astype(bf16) for b in range(B)]
    wq_h = [np.ascontiguousarray(
        wq[:, g * GQ:(g + 1) * GQ].reshape(NKO, 128, GQ).transpose(1, 0, 2)
    ).astype(bf16) for g in range(NKV)]
    wkv_h = [np.ascontiguousarray(
        np.concatenate([wk[:, g * HD:(g + 1) * HD],
                        wv[:, g * HD:(g + 1) * HD]], axis=1)
        .reshape(NKO, 128, 2 * HD).transpose(1, 0, 2)
    ).astype(bf16) for g in range(NKV)]
    for b in range(B):
        for g in range(NKV):
            in_maps.append({
                "dataT": dTs[b],
                "wq": wq_h[g],
                "wkv": wkv_h[g],
                "wo": np.ascontiguousarray(wo[g * GQ:(g + 1) * GQ, :]).astype(bf16),
                "cosT": consts["cosT"],
                "sinT": consts["sinT"],
                "ctd": consts["ctd"],
                "sgn": consts["sgn"],
                "rot": consts["rot"],
                "tril": consts["tril"],
                "ones": consts["ones"],
                "ident": consts["ident"],
            })

    res = run_bass_kernel_spmd(nc, in_maps, core_ids=list(range(8)))
    LAST_RESULTS = res

    out = np.empty((B, S, D), dtype=np.float32)
    for b in range(B):
        acc = res.results[b * NKV]["outT"].astype(np.float32).copy()
        for g in range(1, NKV):
            acc += res.results[b * NKV + g]["outT"]
        out[b] = acc.T
    return out



# revision 55
# speedup vs baseline: 1.0516x; 1.0516x over previous
"""Trainium2 Bass kernel for nn_Attention_197568495719.

Full attention layer: QKV projection + RoPE + int8 KV quant-dequant + GQA
causal SDPA + output projection.  B=2, S=2048, D=2048, 16 q heads / 4 kv
heads, head_dim=128.

Sharding: 8 cores = 2 (batch) x 4 (kv-head groups).  Core (b, g) computes
batch b with q heads 4g..4g+3 and kv head g (tensor parallel on heads:
wq/wk/wv split on output dim, wo on input dim).  Each core produces a
partial outT = (attn @ wo_g).T in [D, S] layout; the host sums the 4
group partials per batch and transposes back.

Design (v3, ~287-304us measured depending on the device's bimodal
clock mode; v2 was ~288-342us, v1 ~485us):
- Everything on the PE is bf16 (hardware fp32r "HIGH" mode multiplies
  with bf16-truncated operands anyway, but pays a ~70ns un-hidden
  fp32 LDWEIGHTS per matmul since FWL is fp32-disabled -- bf16 is
  numerically equivalent and strictly faster).  Host pre-arranges
  dataT/wq/wkv partition-major so every DMA element is >=4KB (512B
  elements run ~3x slower, and small-element descriptors starve their
  whole queue at the packet-round-robin arbiter -- tiny consts ride at
  queue tails).  Initial loads fan out over all three DMA-capable
  queues (sync/gpsimd/scalar) ordered by first use; wo loads ride the
  idle mid-phase-1 DMA window.  A bf16 scratch-matmul burst (512- then
  128-col) bridges the DMA-bound head so the HAM clock-gate never
  re-throttles before real work arrives.
- Phase 1 (projections): 512-token chunks; k/v projected directly into
  [token, dim] tiles so the int8 quant path needs no PE transposes in;
  k RoPE runs along the free axis with a sign-folded sin table; q RoPE
  in place per (head, chunk) with rot matmuls deferred behind all four
  head projections.  Quant rounding uses the fp32 +-1.5*2^23 magic-add
  (exact round-half-to-even, matching jnp.round).
- Phase 2 (attention): ki tiles processed in units of 2 with
  [128,2,QC] wide tiles spanning 2 PSUM banks: one ACT exp per unit
  (amortizing the 352-cycle ACT pipeline fill; ACT is the co-critical
  engine -- diagonal units score 128 extra masked-never-read columns
  on their second half so the whole unit exps in one instruction), one
  wide DVE staging copy per pair, wide finalize ops.  Scores race
  LAG=2 units ahead of the accumulating matmuls.  Causal masking
  multiplies only the 128x128 triangular block per diagonal tile on
  GPSIMD; the accumulating matmuls are trimmed to the exact live
  q-range (128j).  The softmax denominator rides the
  PE as ones-matmuls; off-diagonal units' two exp tiles are pre-summed
  element-wise off the PE (chain-free, alternating GPSIMD/DVE by
  stream) so one ones-matmul covers both ki tiles.  (Fully chained
  engine-side accumulation and DVE reciprocal were both tried and
  measured slower: the chains starve the PE, and DVE reciprocal costs
  ~4.3us per [128,2,512].)  1/Z = exp(-ln(Z)) on ACT: Ln/Exp share an
  ACT function table so no ACT_TABLE_LOAD splits the exp stream; each
  pair's finalize is deferred into the next pair (Ln at pair start
  frees the PSUM slot for reuse).  Each chunk's output projection,
  with each head's q-rope staggered one projection behind its PSUM
  copy and the kq transposes after all projections (clear of the DVE
  quant chain), runs as a dense
  software-pipelined PE block at the next chunk boundary -- all 8 PSUM
  banks are free there, the ACT engine gets a breather between
  exp-heavy pairs, and the first block unit rides the attn slot so it
  never waits on the exp backlog.  outT partials are stored bf16 in dt
  pairs (one wide cast alternating DVE/ACT, one store alternating
  sync/gpsimd); the host accumulates the 4 head-group partials in
  fp32.
"""

import numpy as np

import bass_rust
import concourse.bass as bass
import concourse.tile as tile
import concourse.mybir as mybir
from concourse.bass_utils import run_bass_kernel_spmd

B, S, D = 2, 2048, 2048
NH, NKV, HD = 16, 4, 128
GQ = 512            # q dims per core (4 heads)
NKO = D // 128      # 16 contraction tiles
PC = 512            # projection/attention chunk width (tokens)
NPC = S // PC       # 4
QC = 512
NQC = S // QC       # 4
MAGIC = float(np.float32(12582912.0))  # 1.5 * 2**23
SM_SCALE = 1.0 / float(np.sqrt(HD))

F32 = mybir.dt.float32
F32R = mybir.dt.float32r
BF16 = mybir.dt.bfloat16
MULT = mybir.AluOpType.mult
ADD = mybir.AluOpType.add
EXP = mybir.ActivationFunctionType.Exp

_CACHE = {}

# retained after each kernel() call so test harnesses can read profiling info
LAST_RESULTS = None


def _split_multi_waits(nc):
    """This walrus build caps sync waits at 1 per instruction.  Hoist extra
    waits onto single-wait NoOps immediately preceding the instruction on
    the same engine (identical semantics: the engine is in-order)."""
    for f in nc.m.functions:
        for bb in f.blocks:
            new = []
            for inst in bb.instructions:
                si = inst.sync_info
                if si is None:
                    new.append(inst)
                    continue
                waits = list(si.on_wait)
                if len(waits) > 1:
                    for k, w in enumerate(waits[:-1]):
                        nop = mybir.InstNoOp(name=f"{inst.name}-w{k}", ins=[], outs=[])
                        nop.engine = inst.engine
                        nop.sync_info = bass_rust.SyncInfo(on_wait=[w], on_update=[])
                        new.append(nop)
                    inst.sync_info = bass_rust.SyncInfo(
                        on_wait=[waits[-1]], on_update=list(si.on_update)
                    )
                new.append(inst)
            bb.instructions = new


def _host_consts():
    theta = 10000.0
    angles = 1.0 / theta ** (np.arange(0, HD, 2, dtype=np.float32) / HD)
    emb = np.outer(np.arange(S, dtype=np.float32), angles)
    emb = np.concatenate([emb, emb], axis=-1)          # [S, HD]
    cos = np.cos(emb).astype(np.float32)               # [S, HD]
    sin = np.sin(emb).astype(np.float32)
    cosT = np.ascontiguousarray(cos.T)                 # [128, S]
    sinT = np.ascontiguousarray(sin.T)

    # [t, d]-layout tables for k rope: [p, t_tile, hd]
    ctd = np.ascontiguousarray(cos.reshape(S // 128, 128, HD).transpose(1, 0, 2))
    std = sin.reshape(S // 128, 128, HD).transpose(1, 0, 2).copy()
    sgn = std.copy()
    sgn[:, :, : HD // 2] = -std[:, :, : HD // 2]       # sign-folded sin
    sgn = np.ascontiguousarray(sgn)

    rot = np.zeros((128, 128), dtype=np.float32)       # lhsT of rotate_half
    for i in range(64):
        rot[i, i + 64] = 1.0
        rot[i + 64, i] = -1.0

    p = np.arange(128)[:, None]
    f = np.arange(128)[None, :]
    tril = (p <= f).astype(np.float32)                 # key p visible to q f

    ones = np.ones((128, 128), dtype=np.float32)
    ident = np.eye(128, dtype=np.float32)
    import ml_dtypes
    bf16 = ml_dtypes.bfloat16
    return {
        "cosT": cosT.astype(bf16), "sinT": sinT.astype(bf16),
        "ctd": ctd.astype(bf16), "sgn": sgn.astype(bf16),
        "rot": rot.astype(bf16), "tril": tril.astype(bf16),
        "ones": ones.astype(bf16), "ident": ident,
    }


def _build_nc():
    nc = bass.Bass("TRN2", target_bir_lowering=False, debug=False)

    # host pre-arranges dataT/wq/wkv into partition-major layouts so every
    # DMA element is >=4KB contiguous (512B elements run ~3x slower)
    dataT = nc.dram_tensor("dataT", [128, NPC, NKO, PC], BF16,
                           kind="ExternalInput").ap()
    wq = nc.dram_tensor("wq", [128, NKO, GQ], BF16, kind="ExternalInput").ap()
    wkv = nc.dram_tensor("wkv", [128, NKO, 2 * HD], BF16,
                         kind="ExternalInput").ap()
    wo = nc.dram_tensor("wo", [GQ, D], BF16, kind="ExternalInput").ap()
    cosT_d = nc.dram_tensor("cosT", [128, S], BF16, kind="ExternalInput").ap()
    sinT_d = nc.dram_tensor("sinT", [128, S], BF16, kind="ExternalInput").ap()
    ctd_d = nc.dram_tensor("ctd", [128, NKO, HD], BF16, kind="ExternalInput").ap()
    sgn_d = nc.dram_tensor("sgn", [128, NKO, HD], BF16, kind="ExternalInput").ap()
    rot_d = nc.dram_tensor("rot", [128, 128], BF16, kind="ExternalInput").ap()
    tril_d = nc.dram_tensor("tril", [128, 128], BF16, kind="ExternalInput").ap()
    ones_d = nc.dram_tensor("ones", [128, 128], BF16, kind="ExternalInput").ap()
    ident_d = nc.dram_tensor("ident", [128, 128], F32R, kind="ExternalInput").ap()
    outT = nc.dram_tensor("outT", [D, S], BF16, kind="ExternalOutput").ap()

    dataT_r = dataT                                          # [128, 4, 16, PC]
    wq_r = wq                                                # [128, 16, 512]
    wkv_r = wkv                                              # [128, 16, 256]
    wo_r = wo.rearrange("(h p) n -> p h n", p=128)           # [128, 4, S]
    outT_p = outT.rearrange("(dt p) t -> p dt t", p=128)     # [128, 16, S]

    from contextlib import ExitStack
    with tile.TileContext(nc) as tc, ExitStack() as stack:
        small_consts = stack.enter_context(tc.tile_pool(name="sconsts", bufs=1))
        rot_sb = small_consts.tile([128, 128], BF16)
        ones_sb = small_consts.tile([128, 128], BF16)
        id_sb = small_consts.tile([128, 128], F32R)
        tril_sb = small_consts.tile([128, 128], BF16)

        persist = stack.enter_context(tc.tile_pool(name="persist", bufs=1))
        xq4 = persist.tile([128, 4, S], BF16, name="xq4")    # roped q, [d, h, t]
        kt4 = persist.tile([128, 4, QC], BF16, name="kt4")   # quant k, [d, g, t]
        v_g = [persist.tile([128, 4, HD], BF16, tag=f"vg{g}", name=f"v_g{g}")
               for g in range(4)]                            # quant v, [t, j, d]
        wo_t = [persist.tile([128, S], BF16, tag=f"wo{h}", name=f"wo{h}")
                for h in range(4)]                           # loaded mid-phase-1

        GRP = 4

        # ---------------- Phase 1: projections + rope + quant ----------------
        with tc.tile_pool(name="p1consts", bufs=1) as p1c, \
             tc.tile_pool(name="wpool", bufs=1) as wpool, \
             tc.tile_pool(name="datapool", bufs=2) as datapool, \
             tc.tile_pool(name="kvstage", bufs=2) as kvstage, \
             tc.tile_pool(name="qtmp", bufs=2) as qtmp, \
             tc.tile_pool(name="t2pool", bufs=3) as t2pool, \
             tc.tile_pool(name="proj_ps", bufs=3, space="PSUM") as proj_ps, \
             tc.tile_pool(name="kv_ps", bufs=2, space="PSUM") as kv_ps, \
             tc.tile_pool(name="rope_ps", bufs=2, space="PSUM") as rope_ps, \
             tc.tile_pool(name="tp_ps", bufs=1, space="PSUM") as tp_ps:
            cos_sb = p1c.tile([128, S], BF16)
            sin_sb = p1c.tile([128, S], BF16)
            ctd_sb = p1c.tile([128, NKO, HD], BF16)
            sgn_sb = p1c.tile([128, NKO, HD], BF16)
            wq_sb = wpool.tile([128, NKO, GQ], BF16)
            wkv_sb = wpool.tile([128, NKO, 2 * HD], BF16)

            dT = {}
            for c in range(2):
                dT[c] = datapool.tile([128, NKO, PC], BF16, tag="dT",
                                      name=f"dT{c}")

            # initial loads across all 3 DMA-capable queues (sync/gpsimd/
            # scalar).  The DMA arbiter round-robins PACKETS across queues,
            # so a queue carrying small-element descriptors gets starved:
            # big 4-8KB-element transfers go first in each queue's FIFO.
            # The first-needed tensors are QUARTERED so the kv projection's
            # ko loop starts on the first 0.5MB (per-region tile deps) and
            # trickles, instead of waiting for whole halves; ctd/cos only
            # feed DVE chains with slack, so dT1 outranks them.
            nc.sync.dma_start(dT[0][:, 0:4], dataT_r[:, 0, 0:4])
            nc.gpsimd.dma_start(dT[0][:, 8:12], dataT_r[:, 0, 8:12])
            nc.scalar.dma_start(wkv_sb[:, 0:4], wkv_r[:, 0:4])
            nc.sync.dma_start(dT[0][:, 4:8], dataT_r[:, 0, 4:8])
            nc.gpsimd.dma_start(dT[0][:, 12:16], dataT_r[:, 0, 12:16])
            nc.scalar.dma_start(wkv_sb[:, 4:8], wkv_r[:, 4:8])
            nc.scalar.dma_start(wkv_sb[:, 8:12], wkv_r[:, 8:12])
            nc.scalar.dma_start(wkv_sb[:, 12:16], wkv_r[:, 12:16])
            nc.sync.dma_start(wq_sb[:, 0:4], wq_r[:, 0:4])
            nc.sync.dma_start(wq_sb[:, 4:8], wq_r[:, 4:8])
            nc.scalar.dma_start(wq_sb[:, 8:12], wq_r[:, 8:12])
            nc.scalar.dma_start(wq_sb[:, 12:16], wq_r[:, 12:16])
            nc.sync.dma_start(rot_sb[:], rot_d[:])
            nc.sync.dma_start(id_sb[:], ident_d[:])
            nc.gpsimd.dma_start(dT[1][:, 8:16], dataT_r[:, 1, 8:16])
            nc.sync.dma_start(dT[1][:, 0:8], dataT_r[:, 1, 0:8])
            nc.gpsimd.dma_start(ctd_sb[:], ctd_d[:])
            nc.gpsimd.dma_start(cos_sb[:], cosT_d[:])
            nc.scalar.dma_start(sgn_sb[:], sgn_d[:])
            nc.scalar.dma_start(sin_sb[:], sinT_d[:])
            nc.sync.dma_start(ones_sb[:], ones_d[:])
            nc.sync.dma_start(tril_sb[:], tril_d[:])

            # PE warm-up/filler: scratch matmuls during the initial DMA wait
            # keep the HAM clock-gate at K=8/8 so real work runs at full
            # clock, and bridge to the kv projection's first data (~17us in)
            # so no >3.4us idle window re-throttles the clock.
            warm = wpool.tile([128, QC], BF16, name="warm_scratch")
            nc.vector.memset(warm[:], 0.0)

            def warm_fill(n, w=QC):
                for _ in range(n):
                    wps = rope_ps.tile([128, QC], F32, tag="pr")
                    nc.tensor.matmul(wps[:, 0:w], warm[:, 0:128], warm[:, 0:w],
                                     start=True, stop=True)

            # a short warm burst plus a narrow (<=128-col granularity) tail
            # bridges toward the first kv data without delaying it; longer
            # bridges were tried and measured slower -- the DMA ramp and
            # launch-barrier timing vary too much run-to-run to tune the
            # coverage, and overshooting delays real work at full clock
            warm_fill(12)
            warm_fill(16, w=128)

            def quant_group(src_ap, dst_ap):
                amax = qtmp.tile([128, GRP, 1], F32, tag="amax")
                scl = qtmp.tile([128, GRP, 1], F32, tag="scl")
                inv = qtmp.tile([128, GRP, 1], F32, tag="inv")
                xs = qtmp.tile([128, GRP, HD], F32, tag="xs")
                nc.vector.tensor_reduce(amax[:], src_ap, mybir.AxisListType.X,
                                        mybir.AluOpType.max,
                                        apply_absolute_value=True)
                nc.vector.tensor_scalar_max(amax[:], amax[:], 1e-8)
                nc.vector.tensor_scalar_mul(scl[:], amax[:], 1.0 / 127.0)
                nc.vector.reciprocal(inv[:], scl[:])
                sclb = scl[:].to_broadcast((128, GRP, HD))
                invb = inv[:].to_broadcast((128, GRP, HD))
                nc.vector.tensor_tensor(xs[:], src_ap, invb, MULT)
                nc.vector.tensor_scalar_add(xs[:], xs[:], MAGIC)
                nc.vector.tensor_scalar_add(xs[:], xs[:], -MAGIC)
                nc.vector.tensor_tensor(dst_ap, xs[:], sclb, MULT)

            for c in range(NPC):
                csl = bass.ts(c, PC)
                if c + 2 < NPC:
                    cb = c + 2
                    t_ = datapool.tile([128, NKO, PC], BF16, tag="dT",
                                       name=f"dT{cb}")
                    dT[cb] = t_
                    eng = nc.gpsimd if cb % 2 else nc.sync
                    eng.dma_start(t_[:, 0:8], dataT_r[:, cb, 0:8])
                    eng.dma_start(t_[:, 8:16], dataT_r[:, cb, 8:16])
                if c == 2:
                    # wo is first needed by the out-projection block at the
                    # first phase-2 chunk boundary; load it mid-phase-1
                    # while the DMA queues are otherwise idle
                    for h in range(4):
                        eng = nc.sync if h % 2 else nc.scalar
                        eng.dma_start(wo_t[h][:], wo_r[:, h])

                # --- k/v projection straight into [t, d] tiles ---
                kv_td = kvstage.tile([128, GRP, 2 * HD], F32, tag="kvtd",
                                     name=f"kvtd{c}")
                for j in range(GRP):
                    pkv = kv_ps.tile([128, 2 * HD], F32, tag="pkv")
                    for ko in range(NKO):
                        nc.tensor.matmul(pkv[:],
                                         dT[c][:, ko, bass.ds(j * 128, 128)],
                                         wkv_sb[:, ko],
                                         start=(ko == 0), stop=(ko == NKO - 1))
                    nc.scalar.copy(kv_td[:, j, :], pkv[:])

                # --- k rope along free axis (sign-folded sin table) ---
                kr = kvstage.tile([128, GRP, HD], F32, tag="kr", name=f"kr{c}")
                t2k = qtmp.tile([128, GRP, HD], F32, tag="t2k")
                tsl = bass.ts(c, GRP)  # 4 token tiles of this group
                nc.vector.tensor_tensor(kr[:], kv_td[:, :, 0:HD],
                                        ctd_sb[:, tsl], MULT)
                nc.vector.tensor_tensor(t2k[:, :, 0:64],
                                        kv_td[:, :, 64:HD],
                                        sgn_sb[:, tsl, 0:64], MULT)
                nc.vector.tensor_tensor(t2k[:, :, 64:HD],
                                        kv_td[:, :, 0:64],
                                        sgn_sb[:, tsl, 64:HD], MULT)
                nc.vector.tensor_tensor(kr[:], kr[:], t2k[:], ADD)

                # --- int8 quant-dequant (k roped, v raw); v rides here too
                # so the chunk's DVE work finishes early: the last chunk's
                # DVE tail otherwise delays the phase-2 pool handover ---
                kq = kvstage.tile([128, GRP, HD], F32R, tag="kq", name=f"kq{c}")
                quant_group(kr[:], kq[:])
                quant_group(kv_td[:, :, HD:], v_g[c][:])

                # --- q projection per head, with each head's rope staggered
                # one projection behind its copy (so the PE never waits on
                # the ACT copy), and the kq transposes last (the DVE quant
                # chain is guaranteed done by then, and kt4 isn't read until
                # phase 2) ---
                def emit_qproj(h):
                    pq = proj_ps.tile([128, QC], F32, tag="pq",
                                      name=f"pq{c}_{h}")
                    for ko in range(NKO):
                        nc.tensor.matmul(pq[:], wq_sb[:, ko, bass.ts(h, 128)],
                                         dT[c][:, ko],
                                         start=(ko == 0), stop=(ko == NKO - 1))
                    nc.scalar.copy(xq4[:, h, csl], pq[:])

                def emit_rope(h):
                    pr = rope_ps.tile([128, QC], F32, tag="pr")
                    nc.tensor.matmul(pr[:], rot_sb[:], xq4[:, h, csl],
                                     start=True, stop=True)
                    t1 = t2pool.tile([128, QC], BF16, tag="t1")
                    t2 = t2pool.tile([128, QC], BF16, tag="t2")
                    nc.vector.tensor_tensor(t1[:], xq4[:, h, csl],
                                            cos_sb[:, csl], MULT)
                    nc.vector.tensor_tensor(t2[:], pr[:], sin_sb[:, csl], MULT)
                    nc.vector.tensor_tensor(xq4[:, h, csl], t1[:], t2[:], ADD)

                emit_qproj(0)
                emit_qproj(1)
                emit_rope(0)
                emit_qproj(2)
                emit_rope(1)
                emit_qproj(3)
                emit_rope(2)
                for j in range(GRP):
                    pt = tp_ps.tile([128, 128], F32R, tag="tp")
                    nc.tensor.transpose(pt[:], kq[:, j, :], id_sb[:])
                    nc.scalar.copy(kt4[:, c, bass.ts(j, 128)], pt[:])
                emit_rope(3)

        # ---------------- Phase 2: attention + output projection ----------------
        # ki tiles are processed in units of 2 with [128,2,QC] "wide" tiles
        # spanning 2 PSUM banks / 2KB-per-partition SBUF spans: one exp per
        # off-diagonal unit (amortizes the 352-cycle ACT pipeline fill), one
        # staging copy / store per unit.  The softmax denominator rides the
        # PE as per-ki ones-matmuls (engine-side accumulation measured ~2x
        # slower and starves the PE with serial chains).  Each chunk's
        # output projection runs as a dense PE block at the next chunk
        # boundary, when all 8 PSUM banks are free and the ACT engine gets
        # a breather between exp-heavy pairs.
        with tc.tile_pool(name="attn_sb", bufs=5) as attn_sb, \
             tc.tile_pool(name="exp_pool", bufs=7) as exp_pool, \
             tc.tile_pool(name="araw", bufs=3) as araw_pool, \
             tc.tile_pool(name="rc4p", bufs=2) as rc_pool, \
             tc.tile_pool(name="psum_sb", bufs=5) as psum_pool, \
             tc.tile_pool(name="outstage", bufs=4) as outstage, \
             tc.tile_pool(name="score_ps", bufs=2, space="PSUM") as score_ps, \
             tc.tile_pool(name="attn_ps", bufs=1, space="PSUM") as attn_ps, \
             tc.tile_pool(name="pss_ps", bufs=1, space="PSUM") as pss_ps:

            def out_proj_block(c_prev, tiles, fin):
                # chunk-boundary block: run the previous pair's softmax
                # finalize, then the whole [D, QC] output projection of
                # chunk c_prev software-pipelined 3 units deep over the 4
                # wide PSUM slots (all free at a chunk boundary); the h0/h1
                # lead covers the finalize chain before h2/h3 need its
                # at-tiles.  pu0 rides the attn slot (freed by the DVE ar2
                # copy) so the block's first matmuls never wait on the last
                # pair's ACT exp backlog that still holds the score slots
                fin()
                pools = [(attn_ps, "pa2"), (score_ps, "ps2"),
                         (score_ps, "ps2"), (pss_ps, "pss2")]
                pos = {}

                def finishp(pu):
                    po2 = pos.pop(pu)
                    for half in range(2):
                        dt_ = 2 * pu + half
                        for h2 in (2, 3):
                            at2, sti = tiles[h2]
                            nc.tensor.matmul(po2[:, half],
                                             wo_t[h2][:, bass.ts(dt_, 128)],
                                             at2[:, sti],
                                             start=False, stop=(h2 == 3))
                    ot2 = outstage.tile([128, 2, QC], BF16, tag="ot")
                    if pu % 2:
                        nc.vector.tensor_copy(ot2[:], po2[:])
                    else:
                        nc.scalar.copy(ot2[:], po2[:])
                    eng = nc.gpsimd if pu % 2 else nc.sync
                    eng.dma_start(outT_p[:, 2 * pu:2 * pu + 2,
                                         bass.ts(c_prev, QC)], ot2[:])

                for pu in range(NKO // 2):
                    pool, tag = pools[pu % 4]
                    po2 = pool.tile([128, 2, QC], F32, tag=tag)
                    pos[pu] = po2
                    for half in range(2):
                        dt_ = 2 * pu + half
                        for h2 in (0, 1):
                            at2, sti = tiles[h2]
                            nc.tensor.matmul(po2[:, half],
                                             wo_t[h2][:, bass.ts(dt_, 128)],
                                             at2[:, sti],
                                             start=(h2 == 0), stop=False)
                    if pu >= 3:
                        finishp(pu - 3)
                for pu in range(NKO // 2 - 3, NKO // 2):
                    finishp(pu)

            LAG = 2  # units the score/exp pipeline leads the pa matmuls by

            def emit_pair(c, hA, hB, attn_tiles, carry_in):
                nki = 4 * (c + 1)
                U = nki // 2
                streams = (hA, hB)
                if carry_in is not None:
                    # previous pair's Ln runs first so its pss slot frees
                    # before this pair's ones-matmuls need it
                    carry_in[0]()
                pa2 = attn_ps.tile([128, 2, QC], F32, tag="pa2",
                                   name=f"pa2_{c}_{hA}")
                pss2 = pss_ps.tile([128, 2, QC], F32, tag="pss2",
                                   name=f"pss2_{c}_{hA}")

                def emit_acc(u, et2s, qo, psms):
                    for st in range(2):
                        if psms[st] is not None:
                            # off-diagonal unit: its two exp tiles were
                            # pre-summed element-wise off the PE, so one
                            # ones-matmul covers both ki tiles
                            nc.tensor.matmul(
                                pss2[:, st], ones_sb[:], psms[st][:],
                                start=(2 * u == 0),
                                stop=(2 * u + 1 == nki - 1))
                        for half in range(2):
                            ki = 2 * u + half
                            q = qo[half]
                            if psms[st] is None:
                                nc.tensor.matmul(
                                    pss2[:, st, q:], ones_sb[:],
                                    et2s[st][:, half, q:],
                                    start=(ki == 0), stop=(ki == nki - 1))
                            nc.tensor.matmul(
                                pa2[:, st, q:], v_g[ki // 4][:, ki % 4],
                                et2s[st][:, half, q:],
                                start=(ki == 0), stop=(ki == nki - 1))

                pending = []
                for u in range(U):
                    k0 = 2 * u
                    diag = k0 >= 4 * c
                    qo = (128 * (k0 - 4 * c), 128 * (k0 + 1 - 4 * c)) \
                        if diag else (0, 0)
                    et2s = []
                    psms = []
                    q0 = qo[0]
                    for st in range(2):
                        h = streams[st]
                        ps2 = score_ps.tile([128, 2, QC], F32, tag="ps2")
                        for half in range(2):
                            # both halves score from q0: the diagonal
                            # half-1 computes 128 extra (masked, never
                            # read) columns so the unit exps as ONE wide
                            # ACT instruction -- ACT is the co-critical
                            # engine, the extra PE columns are cheap
                            nc.tensor.matmul(
                                ps2[:, half, q0:],
                                kt4[:, (k0 + half) // 4,
                                    bass.ts((k0 + half) % 4, 128)],
                                xq4[:, h, bass.ds(c * QC + q0, QC - q0)],
                                start=True, stop=True)
                        et2 = exp_pool.tile([128, 2, QC], BF16, tag="et2")
                        et2s.append(et2)
                        nc.scalar.activation(et2[:, :, q0:], ps2[:, :, q0:],
                                             EXP, scale=SM_SCALE)
                        if diag:
                            for half in range(2):
                                q = qo[half]
                                nc.gpsimd.tensor_tensor(
                                    et2[:, half, q:q + 128],
                                    et2[:, half, q:q + 128],
                                    tril_sb[:], MULT)
                            psms.append(None)
                        else:
                            # chain-free pairwise sum of the unit's two exp
                            # tiles (alternating engines by stream) halves
                            # the denominator's PE ones-matmul columns; it
                            # has LAG units of slack before emit_acc reads it
                            psm = psum_pool.tile([128, QC], BF16, tag="psm")
                            peng = nc.gpsimd if st == 0 else nc.vector
                            peng.tensor_tensor(psm[:], et2[:, 0], et2[:, 1],
                                               ADD)
                            psms.append(psm)
                    pending.append((u, et2s, qo, psms))
                    if u >= LAG:
                        emit_acc(*pending.pop(0))
                    if u == 1 and carry_in is not None:
                        carry_in[1]()
                for item in pending:
                    emit_acc(*item)
                # stage the attention accumulator out of PSUM (one wide copy)
                ar2 = araw_pool.tile([128, 2, QC], F32, tag="araw",
                                     name=f"ar2_{c}_{hA}")
                nc.vector.tensor_copy(ar2[:], pa2[:])

                # 1/Z = exp(-ln(Z)) on ACT: Ln and Exp share an ACT function
                # table, so no ACT_TABLE_LOAD ever splits the exp stream,
                # and at ~2.4us the pair is far cheaper than a DVE
                # reciprocal (~4.3us for [128,2,512] -- measured).  fin_a
                # (Ln, reading the PSUM accumulator directly) runs at the
                # next pair's start; fin_b at its second unit.
                state = {}

                def fin_a():
                    lnt = rc_pool.tile([128, 2, QC], F32, tag="lnt")
                    state["lnt"] = lnt
                    nc.scalar.activation(lnt[:], pss2[:],
                                         mybir.ActivationFunctionType.Ln)

                def fin_b():
                    rc2 = rc_pool.tile([128, 2, QC], F32, tag="rc4")
                    nc.scalar.activation(rc2[:], state["lnt"][:], EXP,
                                         scale=-1.0)
                    at2 = attn_sb.tile([128, 2, QC], BF16, tag="attnT")
                    # per-stream multiplies: stream 0's at-tile lands ~0.6us
                    # earlier, unblocking the out-proj block's h2 matmuls
                    for st in range(2):
                        nc.vector.tensor_tensor(at2[:, st], ar2[:, st],
                                                rc2[:, st], MULT)
                        attn_tiles[streams[st]] = (at2, st)

                def fin_tail():
                    fin_a()
                    fin_b()
                return fin_a, fin_b, fin_tail

            prev = None
            for c in range(NQC):
                attn_tiles = {}
                if prev is not None:
                    out_proj_block(prev[0], prev[1], prev[2])
                carry = emit_pair(c, 0, 1, attn_tiles, None)
                carry = emit_pair(c, 2, 3, attn_tiles, carry)
                prev = (c, attn_tiles, carry[2])
            out_proj_block(prev[0], prev[1], prev[2])

    _split_multi_waits(nc)
    return nc


def _get_state():
    if "nc" not in _CACHE:
        _CACHE["nc"] = _build_nc()
        _CACHE["consts"] = _host_consts()
    return _CACHE["nc"], _CACHE["consts"]


def kernel(data=None, mask=None, wq=None, wk=None, wv=None, wo=None, **extra):
    global LAST_RESULTS
    import ml_dtypes
    bf16 = ml_dtypes.bfloat16
    nc, consts = _get_state()

    data = np.asarray(data, dtype=np.float32)
    wq = np.asarray(wq, dtype=np.float32)
    wk = np.asarray(wk, dtype=np.float32)
    wv = np.asarray(wv, dtype=np.float32)
    wo = np.asarray(wo, dtype=np.float32)

    in_maps = []
    # dataT host layout [128, chunk, ko, t]: every DMA element is >=1KB and
    # per-(partition, chunk) spans are 16KB contiguous
    dTs = [np.ascontiguousarray(
        data[b].T.reshape(NKO, 128, NPC, PC).transpose(1, 2, 0, 3)
    ).astype(bf16) for b in range(B)]
    wq_h = [np.ascontiguousarray(
        wq[:, g * GQ:(g + 1) * GQ].reshape(NKO, 128, GQ).transpose(1, 0, 2)
    ).astype(bf16) for g in range(NKV)]
    wkv_h = [np.ascontiguousarray(
        np.concatenate([wk[:, g * HD:(g + 1) * HD],
                        wv[:, g * HD:(g + 1) * HD]], axis=1)
        .reshape(NKO, 128, 2 * HD).transpose(1, 0, 2)
    ).astype(bf16) for g in range(NKV)]
    for b in range(B):
        for g in range(NKV):
            in_maps.append({
                "dataT": dTs[b],
                "wq": wq_h[g],
                "wkv": wkv_h[g],
                "wo": np.ascontiguousarray(wo[g * GQ:(g + 1) * GQ, :]).astype(bf16),
                "cosT": consts["cosT"],
                "sinT": consts["sinT"],
                "ctd": consts["ctd"],
                "sgn": consts["sgn"],
                "rot": consts["rot"],
                "tril": consts["tril"],
                "ones": consts["ones"],
                "ident": consts["ident"],
            })

    res = run_bass_kernel_spmd(nc, in_maps, core_ids=list(range(8)))
    LAST_RESULTS = res

    out = np.empty((B, S, D), dtype=np.float32)
    for b in range(B):
        acc = res.results[b * NKV]["outT"].astype(np.float32).copy()
        for g in range(1, NKV):
            acc += res.results[b * NKV + g]["outT"]
        out[b] = acc.T
    return out



# revision 59
# speedup vs baseline: 1.0574x; 1.0056x over previous
"""Trainium2 Bass kernel for nn_Attention_197568495719.

Full attention layer: QKV projection + RoPE + int8 KV quant-dequant + GQA
causal SDPA + output projection.  B=2, S=2048, D=2048, 16 q heads / 4 kv
heads, head_dim=128.

Sharding: 8 cores = 2 (batch) x 4 (kv-head groups).  Core (b, g) computes
batch b with q heads 4g..4g+3 and kv head g (tensor parallel on heads:
wq/wk/wv split on output dim, wo on input dim).  Each core produces a
partial outT = (attn @ wo_g).T in [D, S] layout; the host sums the 4
group partials per batch and transposes back.

Design (v3, ~287-304us measured depending on the device's bimodal
clock mode; v2 was ~288-342us, v1 ~485us):
- Everything on the PE is bf16 (hardware fp32r "HIGH" mode multiplies
  with bf16-truncated operands anyway, but pays a ~70ns un-hidden
  fp32 LDWEIGHTS per matmul since FWL is fp32-disabled -- bf16 is
  numerically equivalent and strictly faster).  Host pre-arranges
  dataT/wq/wkv partition-major so every DMA element is >=4KB (512B
  elements run ~3x slower, and small-element descriptors starve their
  whole queue at the packet-round-robin arbiter -- tiny consts ride at
  queue tails).  Initial loads fan out over all three DMA-capable
  queues (sync/gpsimd/scalar) ordered by first use; wo loads ride the
  idle mid-phase-1 DMA window.  A bf16 scratch-matmul burst (512- then
  128-col) bridges the DMA-bound head so the HAM clock-gate never
  re-throttles before real work arrives.
- Phase 1 (projections): 512-token chunks; k/v projected directly into
  [token, dim] tiles so the int8 quant path needs no PE transposes in;
  k RoPE runs along the free axis with a sign-folded sin table; q RoPE
  in place per (head, chunk) with rot matmuls deferred behind all four
  head projections.  Quant rounding uses the fp32 +-1.5*2^23 magic-add
  (exact round-half-to-even, matching jnp.round).
- Phase 2 (attention): ki tiles processed in units of 2 with
  [128,2,QC] wide tiles spanning 2 PSUM banks: one ACT exp per unit
  (amortizing the 352-cycle ACT pipeline fill; ACT is the co-critical
  engine -- diagonal units score 128 extra masked-never-read columns
  on their second half so the whole unit exps in one instruction), one
  wide DVE staging copy per pair, wide finalize ops.  Scores race
  LAG=2 units ahead of the accumulating matmuls.  Causal masking
  multiplies only the 128x128 triangular block per diagonal tile on
  GPSIMD; the accumulating matmuls are trimmed to the exact live
  q-range (128j).  The softmax denominator rides the
  PE as ones-matmuls; off-diagonal units' two exp tiles are pre-summed
  element-wise off the PE (chain-free, alternating GPSIMD/DVE by
  stream) so one ones-matmul covers both ki tiles.  (Fully chained
  engine-side accumulation and DVE reciprocal were both tried and
  measured slower: the chains starve the PE, and DVE reciprocal costs
  ~4.3us per [128,2,512].)  1/Z = exp(-ln(Z)) on ACT: Ln/Exp share an
  ACT function table so no ACT_TABLE_LOAD splits the exp stream; each
  pair's finalize is deferred into the next pair (Ln at pair start
  frees the PSUM slot for reuse).  Each chunk's output projection,
  with each head's q-rope staggered one projection behind its PSUM
  copy and the kq transposes after all projections (clear of the DVE
  quant chain), runs as a dense
  software-pipelined PE block at the next chunk boundary -- all 8 PSUM
  banks are free there, the ACT engine gets a breather between
  exp-heavy pairs, and the first block unit rides the attn slot so it
  never waits on the exp backlog.  outT partials are stored bf16 in dt
  pairs (one wide cast alternating DVE/ACT, one store alternating
  sync/gpsimd); the host accumulates the 4 head-group partials in
  fp32.
"""

import numpy as np

import bass_rust
import concourse.bass as bass
import concourse.tile as tile
import concourse.mybir as mybir
from concourse.bass_utils import run_bass_kernel_spmd

B, S, D = 2, 2048, 2048
NH, NKV, HD = 16, 4, 128
GQ = 512            # q dims per core (4 heads)
NKO = D // 128      # 16 contraction tiles
PC = 512            # projection/attention chunk width (tokens)
NPC = S // PC       # 4
QC = 512
NQC = S // QC       # 4
MAGIC = float(np.float32(12582912.0))  # 1.5 * 2**23
SM_SCALE = 1.0 / float(np.sqrt(HD))

F32 = mybir.dt.float32
F32R = mybir.dt.float32r
BF16 = mybir.dt.bfloat16
MULT = mybir.AluOpType.mult
ADD = mybir.AluOpType.add
EXP = mybir.ActivationFunctionType.Exp

_CACHE = {}

# retained after each kernel() call so test harnesses can read profiling info
LAST_RESULTS = None


def _split_multi_waits(nc):
    """This walrus build caps sync waits at 1 per instruction.  Hoist extra
    waits onto single-wait NoOps immediately preceding the instruction on
    the same engine (identical semantics: the engine is in-order)."""
    for f in nc.m.functions:
        for bb in f.blocks:
            new = []
            for inst in bb.instructions:
                si = inst.sync_info
                if si is None:
                    new.append(inst)
                    continue
                waits = list(si.on_wait)
                if len(waits) > 1:
                    for k, w in enumerate(waits[:-1]):
                        nop = mybir.InstNoOp(name=f"{inst.name}-w{k}", ins=[], outs=[])
                        nop.engine = inst.engine
                        nop.sync_info = bass_rust.SyncInfo(on_wait=[w], on_update=[])
                        new.append(nop)
                    inst.sync_info = bass_rust.SyncInfo(
                        on_wait=[waits[-1]], on_update=list(si.on_update)
                    )
                new.append(inst)
            bb.instructions = new


def _host_consts():
    theta = 10000.0
    angles = 1.0 / theta ** (np.arange(0, HD, 2, dtype=np.float32) / HD)
    emb = np.outer(np.arange(S, dtype=np.float32), angles)
    emb = np.concatenate([emb, emb], axis=-1)          # [S, HD]
    cos = np.cos(emb).astype(np.float32)               # [S, HD]
    sin = np.sin(emb).astype(np.float32)
    cosT = np.ascontiguousarray(cos.T)                 # [128, S]
    sinT = np.ascontiguousarray(sin.T)

    # [t, d]-layout tables for k rope: [p, t_tile, hd]
    ctd = np.ascontiguousarray(cos.reshape(S // 128, 128, HD).transpose(1, 0, 2))
    std = sin.reshape(S // 128, 128, HD).transpose(1, 0, 2).copy()
    sgn = std.copy()
    sgn[:, :, : HD // 2] = -std[:, :, : HD // 2]       # sign-folded sin
    sgn = np.ascontiguousarray(sgn)

    rot = np.zeros((128, 128), dtype=np.float32)       # lhsT of rotate_half
    for i in range(64):
        rot[i, i + 64] = 1.0
        rot[i + 64, i] = -1.0

    p = np.arange(128)[:, None]
    f = np.arange(128)[None, :]
    tril = (p <= f).astype(np.float32)                 # key p visible to q f

    ones = np.ones((128, 128), dtype=np.float32)
    ident = np.eye(128, dtype=np.float32)
    import ml_dtypes
    bf16 = ml_dtypes.bfloat16
    return {
        "cosT": cosT.astype(bf16), "sinT": sinT.astype(bf16),
        "ctd": ctd.astype(bf16), "sgn": sgn.astype(bf16),
        "rot": rot.astype(bf16), "tril": tril.astype(bf16),
        "ones": ones.astype(bf16), "ident": ident,
    }


def _build_nc():
    nc = bass.Bass("TRN2", target_bir_lowering=False, debug=False)

    # host pre-arranges dataT/wq/wkv into partition-major layouts so every
    # DMA element is >=4KB contiguous (512B elements run ~3x slower)
    dataT = nc.dram_tensor("dataT", [128, NPC, NKO, PC], BF16,
                           kind="ExternalInput").ap()
    wq = nc.dram_tensor("wq", [128, NKO, GQ], BF16, kind="ExternalInput").ap()
    wkv = nc.dram_tensor("wkv", [128, NKO, 2 * HD], BF16,
                         kind="ExternalInput").ap()
    wo = nc.dram_tensor("wo", [GQ, D], BF16, kind="ExternalInput").ap()
    cosT_d = nc.dram_tensor("cosT", [128, S], BF16, kind="ExternalInput").ap()
    sinT_d = nc.dram_tensor("sinT", [128, S], BF16, kind="ExternalInput").ap()
    ctd_d = nc.dram_tensor("ctd", [128, NKO, HD], BF16, kind="ExternalInput").ap()
    sgn_d = nc.dram_tensor("sgn", [128, NKO, HD], BF16, kind="ExternalInput").ap()
    rot_d = nc.dram_tensor("rot", [128, 128], BF16, kind="ExternalInput").ap()
    tril_d = nc.dram_tensor("tril", [128, 128], BF16, kind="ExternalInput").ap()
    ones_d = nc.dram_tensor("ones", [128, 128], BF16, kind="ExternalInput").ap()
    ident_d = nc.dram_tensor("ident", [128, 128], F32R, kind="ExternalInput").ap()
    outT = nc.dram_tensor("outT", [D, S], BF16, kind="ExternalOutput").ap()

    dataT_r = dataT                                          # [128, 4, 16, PC]
    wq_r = wq                                                # [128, 16, 512]
    wkv_r = wkv                                              # [128, 16, 256]
    wo_r = wo.rearrange("(h p) n -> p h n", p=128)           # [128, 4, S]
    outT_p = outT.rearrange("(dt p) t -> p dt t", p=128)     # [128, 16, S]

    from contextlib import ExitStack
    with tile.TileContext(nc) as tc, ExitStack() as stack:
        small_consts = stack.enter_context(tc.tile_pool(name="sconsts", bufs=1))
        rot_sb = small_consts.tile([128, 128], BF16)
        ones_sb = small_consts.tile([128, 128], BF16)
        id_sb = small_consts.tile([128, 128], F32R)
        tril_sb = small_consts.tile([128, 128], BF16)

        persist = stack.enter_context(tc.tile_pool(name="persist", bufs=1))
        xq4 = persist.tile([128, 4, S], BF16, name="xq4")    # roped q, [d, h, t]
        kt4 = persist.tile([128, 4, QC], BF16, name="kt4")   # quant k, [d, g, t]
        v_g = [persist.tile([128, 4, HD], BF16, tag=f"vg{g}", name=f"v_g{g}")
               for g in range(4)]                            # quant v, [t, j, d]
        wo_t = [persist.tile([128, S], BF16, tag=f"wo{h}", name=f"wo{h}")
                for h in range(4)]                           # loaded mid-phase-1

        GRP = 4

        # ---------------- Phase 1: projections + rope + quant ----------------
        with tc.tile_pool(name="p1consts", bufs=1) as p1c, \
             tc.tile_pool(name="wpool", bufs=1) as wpool, \
             tc.tile_pool(name="datapool", bufs=2) as datapool, \
             tc.tile_pool(name="kvstage", bufs=2) as kvstage, \
             tc.tile_pool(name="qtmp", bufs=2) as qtmp, \
             tc.tile_pool(name="t2pool", bufs=3) as t2pool, \
             tc.tile_pool(name="proj_ps", bufs=3, space="PSUM") as proj_ps, \
             tc.tile_pool(name="kv_ps", bufs=2, space="PSUM") as kv_ps, \
             tc.tile_pool(name="rope_ps", bufs=2, space="PSUM") as rope_ps, \
             tc.tile_pool(name="tp_ps", bufs=1, space="PSUM") as tp_ps:
            cos_sb = p1c.tile([128, S], BF16)
            sin_sb = p1c.tile([128, S], BF16)
            ctd_sb = p1c.tile([128, NKO, HD], BF16)
            sgn_sb = p1c.tile([128, NKO, HD], BF16)
            wq_sb = wpool.tile([128, NKO, GQ], BF16)
            wkv_sb = wpool.tile([128, NKO, 2 * HD], BF16)

            dT = {}
            for c in range(2):
                dT[c] = datapool.tile([128, NKO, PC], BF16, tag="dT",
                                      name=f"dT{c}")

            # initial loads across all 3 DMA-capable queues (sync/gpsimd/
            # scalar).  The DMA arbiter round-robins PACKETS across queues,
            # so a queue carrying small-element descriptors gets starved:
            # big 4-8KB-element transfers go first in each queue's FIFO.
            # The first-needed tensors are QUARTERED so the kv projection's
            # ko loop starts on the first 0.5MB (per-region tile deps) and
            # trickles, instead of waiting for whole halves; ctd/cos only
            # feed DVE chains with slack, so dT1 outranks them.
            nc.sync.dma_start(dT[0][:, 0:4], dataT_r[:, 0, 0:4])
            nc.gpsimd.dma_start(dT[0][:, 8:12], dataT_r[:, 0, 8:12])
            nc.scalar.dma_start(wkv_sb[:, 0:4], wkv_r[:, 0:4])
            nc.sync.dma_start(dT[0][:, 4:8], dataT_r[:, 0, 4:8])
            nc.gpsimd.dma_start(dT[0][:, 12:16], dataT_r[:, 0, 12:16])
            nc.scalar.dma_start(wkv_sb[:, 4:8], wkv_r[:, 4:8])
            nc.scalar.dma_start(wkv_sb[:, 8:12], wkv_r[:, 8:12])
            nc.scalar.dma_start(wkv_sb[:, 12:16], wkv_r[:, 12:16])
            nc.sync.dma_start(wq_sb[:, 0:4], wq_r[:, 0:4])
            nc.sync.dma_start(wq_sb[:, 4:8], wq_r[:, 4:8])
            nc.scalar.dma_start(wq_sb[:, 8:12], wq_r[:, 8:12])
            nc.scalar.dma_start(wq_sb[:, 12:16], wq_r[:, 12:16])
            nc.sync.dma_start(rot_sb[:], rot_d[:])
            nc.sync.dma_start(id_sb[:], ident_d[:])
            nc.gpsimd.dma_start(dT[1][:, 8:16], dataT_r[:, 1, 8:16])
            nc.sync.dma_start(dT[1][:, 0:8], dataT_r[:, 1, 0:8])
            nc.gpsimd.dma_start(ctd_sb[:], ctd_d[:])
            nc.gpsimd.dma_start(cos_sb[:], cosT_d[:])
            nc.scalar.dma_start(sgn_sb[:], sgn_d[:])
            nc.scalar.dma_start(sin_sb[:], sinT_d[:])
            nc.sync.dma_start(ones_sb[:], ones_d[:])
            nc.sync.dma_start(tril_sb[:], tril_d[:])

            # PE warm-up/filler: scratch matmuls during the initial DMA wait
            # keep the HAM clock-gate at K=8/8 so real work runs at full
            # clock, and bridge to the kv projection's first data (~17us in)
            # so no >3.4us idle window re-throttles the clock.
            warm = wpool.tile([128, QC], BF16, name="warm_scratch")
            nc.vector.memset(warm[:], 0.0)

            def warm_fill(n, w=QC):
                for _ in range(n):
                    wps = rope_ps.tile([128, QC], F32, tag="pr")
                    nc.tensor.matmul(wps[:, 0:w], warm[:, 0:128], warm[:, 0:w],
                                     start=True, stop=True)

            # a short warm burst plus a narrow (<=128-col granularity) tail
            # bridges toward the first kv data without delaying it; longer
            # bridges were tried and measured slower -- the DMA ramp and
            # launch-barrier timing vary too much run-to-run to tune the
            # coverage, and overshooting delays real work at full clock
            warm_fill(12)
            warm_fill(16, w=128)

            def quant_group(src_ap, dst_ap):
                amax = qtmp.tile([128, GRP, 1], F32, tag="amax")
                scl = qtmp.tile([128, GRP, 1], F32, tag="scl")
                inv = qtmp.tile([128, GRP, 1], F32, tag="inv")
                xs = qtmp.tile([128, GRP, HD], F32, tag="xs")
                nc.vector.tensor_reduce(amax[:], src_ap, mybir.AxisListType.X,
                                        mybir.AluOpType.max,
                                        apply_absolute_value=True)
                nc.vector.tensor_scalar_max(amax[:], amax[:], 1e-8)
                nc.vector.tensor_scalar_mul(scl[:], amax[:], 1.0 / 127.0)
                nc.vector.reciprocal(inv[:], scl[:])
                sclb = scl[:].to_broadcast((128, GRP, HD))
                invb = inv[:].to_broadcast((128, GRP, HD))
                nc.vector.tensor_tensor(xs[:], src_ap, invb, MULT)
                nc.vector.tensor_scalar_add(xs[:], xs[:], MAGIC)
                nc.vector.tensor_scalar_add(xs[:], xs[:], -MAGIC)
                nc.vector.tensor_tensor(dst_ap, xs[:], sclb, MULT)

            for c in range(NPC):
                csl = bass.ts(c, PC)
                if c + 2 < NPC:
                    cb = c + 2
                    t_ = datapool.tile([128, NKO, PC], BF16, tag="dT",
                                       name=f"dT{cb}")
                    dT[cb] = t_
                    eng = nc.gpsimd if cb % 2 else nc.sync
                    eng.dma_start(t_[:, 0:8], dataT_r[:, cb, 0:8])
                    eng.dma_start(t_[:, 8:16], dataT_r[:, cb, 8:16])
                if c == 2:
                    # wo is first needed by the out-projection block at the
                    # first phase-2 chunk boundary; load it mid-phase-1
                    # while the DMA queues are otherwise idle
                    for h in range(4):
                        eng = nc.sync if h % 2 else nc.scalar
                        eng.dma_start(wo_t[h][:], wo_r[:, h])

                # --- k/v projection straight into [t, d] tiles ---
                kv_td = kvstage.tile([128, GRP, 2 * HD], F32, tag="kvtd",
                                     name=f"kvtd{c}")
                for j in range(GRP):
                    pkv = kv_ps.tile([128, 2 * HD], F32, tag="pkv")
                    for ko in range(NKO):
                        nc.tensor.matmul(pkv[:],
                                         dT[c][:, ko, bass.ds(j * 128, 128)],
                                         wkv_sb[:, ko],
                                         start=(ko == 0), stop=(ko == NKO - 1))
                    nc.scalar.copy(kv_td[:, j, :], pkv[:])

                # --- k rope along free axis (sign-folded sin table) ---
                kr = kvstage.tile([128, GRP, HD], F32, tag="kr", name=f"kr{c}")
                t2k = qtmp.tile([128, GRP, HD], F32, tag="t2k")
                tsl = bass.ts(c, GRP)  # 4 token tiles of this group
                nc.vector.tensor_tensor(kr[:], kv_td[:, :, 0:HD],
                                        ctd_sb[:, tsl], MULT)
                nc.vector.tensor_tensor(t2k[:, :, 0:64],
                                        kv_td[:, :, 64:HD],
                                        sgn_sb[:, tsl, 0:64], MULT)
                nc.vector.tensor_tensor(t2k[:, :, 64:HD],
                                        kv_td[:, :, 0:64],
                                        sgn_sb[:, tsl, 64:HD], MULT)
                nc.vector.tensor_tensor(kr[:], kr[:], t2k[:], ADD)

                # --- int8 quant-dequant (k roped, v raw); v rides here too
                # so the chunk's DVE work finishes early: the last chunk's
                # DVE tail otherwise delays the phase-2 pool handover ---
                kq = kvstage.tile([128, GRP, HD], F32R, tag="kq", name=f"kq{c}")
                quant_group(kr[:], kq[:])
                quant_group(kv_td[:, :, HD:], v_g[c][:])

                # --- q projection per head, with each head's rope staggered
                # one projection behind its copy (so the PE never waits on
                # the ACT copy), and the kq transposes last (the DVE quant
                # chain is guaranteed done by then, and kt4 isn't read until
                # phase 2) ---
                def emit_qproj(h):
                    pq = proj_ps.tile([128, QC], F32, tag="pq",
                                      name=f"pq{c}_{h}")
                    for ko in range(NKO):
                        nc.tensor.matmul(pq[:], wq_sb[:, ko, bass.ts(h, 128)],
                                         dT[c][:, ko],
                                         start=(ko == 0), stop=(ko == NKO - 1))
                    nc.scalar.copy(xq4[:, h, csl], pq[:])

                def emit_rope(h):
                    pr = rope_ps.tile([128, QC], F32, tag="pr")
                    nc.tensor.matmul(pr[:], rot_sb[:], xq4[:, h, csl],
                                     start=True, stop=True)
                    t1 = t2pool.tile([128, QC], BF16, tag="t1")
                    t2 = t2pool.tile([128, QC], BF16, tag="t2")
                    nc.vector.tensor_tensor(t1[:], xq4[:, h, csl],
                                            cos_sb[:, csl], MULT)
                    nc.vector.tensor_tensor(t2[:], pr[:], sin_sb[:, csl], MULT)
                    nc.vector.tensor_tensor(xq4[:, h, csl], t1[:], t2[:], ADD)

                emit_qproj(0)
                emit_qproj(1)
                emit_rope(0)
                emit_qproj(2)
                emit_rope(1)
                emit_qproj(3)
                emit_rope(2)
                for j in range(GRP):
                    pt = tp_ps.tile([128, 128], F32R, tag="tp")
                    nc.tensor.transpose(pt[:], kq[:, j, :], id_sb[:])
                    nc.scalar.copy(kt4[:, c, bass.ts(j, 128)], pt[:])
                emit_rope(3)

        # ---------------- Phase 2: attention + output projection ----------------
        # ki tiles are processed in units of 2 with [128,2,QC] "wide" tiles
        # spanning 2 PSUM banks / 2KB-per-partition SBUF spans: one exp per
        # off-diagonal unit (amortizes the 352-cycle ACT pipeline fill), one
        # staging copy / store per unit.  The softmax denominator rides the
        # PE as per-ki ones-matmuls (engine-side accumulation measured ~2x
        # slower and starves the PE with serial chains).  Each chunk's
        # output projection runs as a dense PE block at the next chunk
        # boundary, when all 8 PSUM banks are free and the ACT engine gets
        # a breather between exp-heavy pairs.
        with tc.tile_pool(name="attn_sb", bufs=5) as attn_sb, \
             tc.tile_pool(name="exp_pool", bufs=7) as exp_pool, \
             tc.tile_pool(name="araw", bufs=3) as araw_pool, \
             tc.tile_pool(name="rc4p", bufs=2) as rc_pool, \
             tc.tile_pool(name="psum_sb", bufs=7) as psum_pool, \
             tc.tile_pool(name="outstage", bufs=4) as outstage, \
             tc.tile_pool(name="score_ps", bufs=2, space="PSUM") as score_ps, \
             tc.tile_pool(name="attn_ps", bufs=1, space="PSUM") as attn_ps, \
             tc.tile_pool(name="pss_ps", bufs=1, space="PSUM") as pss_ps:

            def out_proj_block(c_prev, tiles, fin):
                # chunk-boundary block: run the previous pair's softmax
                # finalize, then the whole [D, QC] output projection of
                # chunk c_prev software-pipelined 3 units deep over the 4
                # wide PSUM slots (all free at a chunk boundary); the h0/h1
                # lead covers the finalize chain before h2/h3 need its
                # at-tiles.  pu0 rides the attn slot (freed by the DVE ar2
                # copy) so the block's first matmuls never wait on the last
                # pair's ACT exp backlog that still holds the score slots
                fin()
                pools = [(attn_ps, "pa2"), (score_ps, "ps2"),
                         (score_ps, "ps2"), (pss_ps, "pss2")]
                pos = {}

                def finishp(pu):
                    po2 = pos.pop(pu)
                    for half in range(2):
                        dt_ = 2 * pu + half
                        for h2 in (2, 3):
                            at2, sti = tiles[h2]
                            nc.tensor.matmul(po2[:, half],
                                             wo_t[h2][:, bass.ts(dt_, 128)],
                                             at2[:, sti],
                                             start=False, stop=(h2 == 3))
                    ot2 = outstage.tile([128, 2, QC], BF16, tag="ot")
                    if pu % 2:
                        nc.vector.tensor_copy(ot2[:], po2[:])
                    else:
                        nc.scalar.copy(ot2[:], po2[:])
                    eng = nc.gpsimd if pu % 2 else nc.sync
                    eng.dma_start(outT_p[:, 2 * pu:2 * pu + 2,
                                         bass.ts(c_prev, QC)], ot2[:])

                for pu in range(NKO // 2):
                    pool, tag = pools[pu % 4]
                    po2 = pool.tile([128, 2, QC], F32, tag=tag)
                    pos[pu] = po2
                    for half in range(2):
                        dt_ = 2 * pu + half
                        for h2 in (0, 1):
                            at2, sti = tiles[h2]
                            nc.tensor.matmul(po2[:, half],
                                             wo_t[h2][:, bass.ts(dt_, 128)],
                                             at2[:, sti],
                                             start=(h2 == 0), stop=False)
                    if pu >= 3:
                        finishp(pu - 3)
                for pu in range(NKO // 2 - 3, NKO // 2):
                    finishp(pu)

            LAG = 2  # units the score/exp pipeline leads the pa matmuls by

            def emit_pair(c, hA, hB, attn_tiles, carry_in):
                nki = 4 * (c + 1)
                U = nki // 2
                streams = (hA, hB)
                if carry_in is not None:
                    # previous pair's Ln runs first so its pss slot frees
                    # before this pair's ones-matmuls need it
                    carry_in[0]()
                pa2 = attn_ps.tile([128, 2, QC], F32, tag="pa2",
                                   name=f"pa2_{c}_{hA}")
                pss2 = pss_ps.tile([128, 2, QC], F32, tag="pss2",
                                   name=f"pss2_{c}_{hA}")

                def emit_acc(u, et2s, qo, psms):
                    diag = qo[1] != 0
                    for st in range(2):
                        if psms[st] is not None:
                            # odd off-diagonal unit: the level-2 pairwise
                            # sum covers THIS unit and the previous one, so
                            # one ones-matmul covers four ki tiles (even
                            # off-diagonal units emit no ones-matmul at all)
                            nc.tensor.matmul(
                                pss2[:, st], ones_sb[:], psms[st][:],
                                start=(u == 1), stop=False)
                        for half in range(2):
                            ki = 2 * u + half
                            q = qo[half]
                            if diag:
                                nc.tensor.matmul(
                                    pss2[:, st, q:], ones_sb[:],
                                    et2s[st][:, half, q:],
                                    start=(ki == 0), stop=(ki == nki - 1))
                            nc.tensor.matmul(
                                pa2[:, st, q:], v_g[ki // 4][:, ki % 4],
                                et2s[st][:, half, q:],
                                start=(ki == 0), stop=(ki == nki - 1))

                pending = []
                last_psm = [None, None]
                for u in range(U):
                    k0 = 2 * u
                    diag = k0 >= 4 * c
                    qo = (128 * (k0 - 4 * c), 128 * (k0 + 1 - 4 * c)) \
                        if diag else (0, 0)
                    et2s = []
                    psms = []
                    q0 = qo[0]
                    for st in range(2):
                        h = streams[st]
                        ps2 = score_ps.tile([128, 2, QC], F32, tag="ps2")
                        for half in range(2):
                            # both halves score from q0: the diagonal
                            # half-1 computes 128 extra (masked, never
                            # read) columns so the unit exps as ONE wide
                            # ACT instruction -- ACT is the co-critical
                            # engine, the extra PE columns are cheap
                            nc.tensor.matmul(
                                ps2[:, half, q0:],
                                kt4[:, (k0 + half) // 4,
                                    bass.ts((k0 + half) % 4, 128)],
                                xq4[:, h, bass.ds(c * QC + q0, QC - q0)],
                                start=True, stop=True)
                        et2 = exp_pool.tile([128, 2, QC], BF16, tag="et2")
                        et2s.append(et2)
                        nc.scalar.activation(et2[:, :, q0:], ps2[:, :, q0:],
                                             EXP, scale=SM_SCALE)
                        if diag:
                            for half in range(2):
                                q = qo[half]
                                nc.gpsimd.tensor_tensor(
                                    et2[:, half, q:q + 128],
                                    et2[:, half, q:q + 128],
                                    tril_sb[:], MULT)
                            psms.append(None)
                        else:
                            # chain-free pairwise sums of the exp tiles
                            # (alternating engines by stream): level 1 sums
                            # the unit's two tiles, level 2 sums consecutive
                            # off-diagonal units, so the PE runs one
                            # ones-matmul per FOUR ki tiles.  Each add has
                            # LAG units of slack before emit_acc reads it.
                            psm = psum_pool.tile([128, QC], BF16, tag="psm")
                            peng = nc.gpsimd if st == 0 else nc.vector
                            peng.tensor_tensor(psm[:], et2[:, 0], et2[:, 1],
                                               ADD)
                            if u % 2 == 0:
                                last_psm[st] = psm
                                psms.append(None)
                            else:
                                psm2 = psum_pool.tile([128, QC], BF16,
                                                      tag="psm")
                                peng.tensor_tensor(psm2[:], last_psm[st][:],
                                                   psm[:], ADD)
                                psms.append(psm2)
                    pending.append((u, et2s, qo, psms))
                    if u >= LAG:
                        emit_acc(*pending.pop(0))
                    if u == 1 and carry_in is not None:
                        carry_in[1]()
                for item in pending:
                    emit_acc(*item)
                # stage the attention accumulator out of PSUM (one wide copy)
                ar2 = araw_pool.tile([128, 2, QC], F32, tag="araw",
                                     name=f"ar2_{c}_{hA}")
                nc.vector.tensor_copy(ar2[:], pa2[:])

                # 1/Z = exp(-ln(Z)) on ACT: Ln and Exp share an ACT function
                # table, so no ACT_TABLE_LOAD ever splits the exp stream,
                # and at ~2.4us the pair is far cheaper than a DVE
                # reciprocal (~4.3us for [128,2,512] -- measured).  fin_a
                # (Ln, reading the PSUM accumulator directly) runs at the
                # next pair's start; fin_b at its second unit.
                state = {}

                def fin_a():
                    lnt = rc_pool.tile([128, 2, QC], F32, tag="lnt")
                    state["lnt"] = lnt
                    nc.scalar.activation(lnt[:], pss2[:],
                                         mybir.ActivationFunctionType.Ln)

                def fin_b():
                    rc2 = rc_pool.tile([128, 2, QC], F32, tag="rc4")
                    nc.scalar.activation(rc2[:], state["lnt"][:], EXP,
                                         scale=-1.0)
                    at2 = attn_sb.tile([128, 2, QC], BF16, tag="attnT")
                    # per-stream multiplies: stream 0's at-tile lands ~0.6us
                    # earlier, unblocking the out-proj block's h2 matmuls
                    for st in range(2):
                        nc.vector.tensor_tensor(at2[:, st], ar2[:, st],
                                                rc2[:, st], MULT)
                        attn_tiles[streams[st]] = (at2, st)

                def fin_tail():
                    fin_a()
                    fin_b()
                return fin_a, fin_b, fin_tail

            prev = None
            for c in range(NQC):
                attn_tiles = {}
                if prev is not None:
                    out_proj_block(prev[0], prev[1], prev[2])
                carry = emit_pair(c, 0, 1, attn_tiles, None)
                carry = emit_pair(c, 2, 3, attn_tiles, carry)
                prev = (c, attn_tiles, carry[2])
            out_proj_block(prev[0], prev[1], prev[2])

    _split_multi_waits(nc)
    return nc


def _get_state():
    if "nc" not in _CACHE:
        _CACHE["nc"] = _build_nc()
        _CACHE["consts"] = _host_consts()
    return _CACHE["nc"], _CACHE["consts"]


def kernel(data=None, mask=None, wq=None, wk=None, wv=None, wo=None, **extra):
    global LAST_RESULTS
    import ml_dtypes
    bf16 = ml_dtypes.bfloat16
    nc, consts = _get_state()

    data = np.asarray(data, dtype=np.float32)
    wq = np.asarray(wq, dtype=np.float32)
    wk = np.asarray(wk, dtype=np.float32)
    wv = np.asarray(wv, dtype=np.float32)
    wo = np.asarray(wo, dtype=np.float32)

    in_maps = []
    # dataT host layout [128, chunk, ko, t]: every DMA element is >=1KB and
    # per-(partition, chunk) spans are 16KB contiguous
    dTs = [np.ascontiguousarray(
        data[b].T.reshape(NKO, 128, NPC, PC).transpose(1, 2, 0, 3)
    ).astype(bf16) for b in range(B)]
    wq_h = [np.ascontiguousarray(
        wq[:, g * GQ:(g + 1) * GQ].reshape(NKO, 128, GQ).transpose(1, 0, 2)
    ).astype(bf16) for g in range(NKV)]
    wkv_h = [np.ascontiguousarray(
        np.concatenate([wk[:, g * HD:(g + 1) * HD],
                        wv[:, g * HD:(g + 1) * HD]], axis=1)
        .reshape(NKO, 128, 2 * HD).transpose(1, 0, 2)
    ).astype(bf16) for g in range(NKV)]
    for b in range(B):
        for g in range(NKV):
            in_maps.append({
                "dataT": dTs[b],
                "wq": wq_h[g],
                "wkv": wkv_h[g],
                "wo": np.ascontiguousarray(wo[g * GQ:(g + 1) * GQ, :]).astype(bf16),
                "cosT": consts["cosT"],
                "sinT": consts["sinT"],
                "ctd": consts["ctd"],
                "sgn": consts["sgn"],
                "rot": consts["rot"],
                "tril": consts["tril"],
                "ones": consts["ones"],
                "ident": consts["ident"],
            })

    res = run_bass_kernel_spmd(nc, in_maps, core_ids=list(range(8)))
    LAST_RESULTS = res

    out = np.empty((B, S, D), dtype=np.float32)
    for b in range(B):
        acc = res.results[b * NKV]["outT"].astype(np.float32).copy()
        for g in range(1, NKV):
            acc += res.results[b * NKV + g]["outT"]
        out[b] = acc.T
    return out



# revision 60
# speedup vs baseline: 1.0634x; 1.0057x over previous
"""Trainium2 Bass kernel for nn_Attention_197568495719.

Full attention layer: QKV projection + RoPE + int8 KV quant-dequant + GQA
causal SDPA + output projection.  B=2, S=2048, D=2048, 16 q heads / 4 kv
heads, head_dim=128.

Sharding: 8 cores = 2 (batch) x 4 (kv-head groups).  Core (b, g) computes
batch b with q heads 4g..4g+3 and kv head g (tensor parallel on heads:
wq/wk/wv split on output dim, wo on input dim).  Each core produces a
partial outT = (attn @ wo_g).T in [D, S] layout; the host sums the 4
group partials per batch and transposes back.

Design (v3, ~287-304us measured depending on the device's bimodal
clock mode; v2 was ~288-342us, v1 ~485us):
- Everything on the PE is bf16 (hardware fp32r "HIGH" mode multiplies
  with bf16-truncated operands anyway, but pays a ~70ns un-hidden
  fp32 LDWEIGHTS per matmul since FWL is fp32-disabled -- bf16 is
  numerically equivalent and strictly faster).  Host pre-arranges
  dataT/wq/wkv partition-major so every DMA element is >=4KB (512B
  elements run ~3x slower, and small-element descriptors starve their
  whole queue at the packet-round-robin arbiter -- tiny consts ride at
  queue tails).  Initial loads fan out over all three DMA-capable
  queues (sync/gpsimd/scalar) ordered by first use; wo loads ride the
  idle mid-phase-1 DMA window.  A bf16 scratch-matmul burst (512- then
  128-col) bridges the DMA-bound head so the HAM clock-gate never
  re-throttles before real work arrives.
- Phase 1 (projections): 512-token chunks; k/v projected directly into
  [token, dim] tiles so the int8 quant path needs no PE transposes in;
  k RoPE runs along the free axis with a sign-folded sin table; q RoPE
  in place per (head, chunk) with rot matmuls deferred behind all four
  head projections.  Quant rounding uses the fp32 +-1.5*2^23 magic-add
  (exact round-half-to-even, matching jnp.round).
- Phase 2 (attention): ki tiles processed in units of 2 with
  [128,2,QC] wide tiles spanning 2 PSUM banks: one ACT exp per unit
  (amortizing the 352-cycle ACT pipeline fill; ACT is the co-critical
  engine -- diagonal units score 128 extra masked-never-read columns
  on their second half so the whole unit exps in one instruction), one
  wide DVE staging copy per pair, wide finalize ops.  Scores race
  LAG=2 units ahead of the accumulating matmuls.  Causal masking
  multiplies only the 128x128 triangular block per diagonal tile on
  GPSIMD; the accumulating matmuls are trimmed to the exact live
  q-range (128j).  The softmax denominator rides the
  PE as ones-matmuls; off-diagonal units' two exp tiles are pre-summed
  element-wise off the PE (chain-free, alternating GPSIMD/DVE by
  stream) so one ones-matmul covers both ki tiles.  (Fully chained
  engine-side accumulation and DVE reciprocal were both tried and
  measured slower: the chains starve the PE, and DVE reciprocal costs
  ~4.3us per [128,2,512].)  1/Z = exp(-ln(Z)) on ACT: Ln/Exp share an
  ACT function table so no ACT_TABLE_LOAD splits the exp stream; each
  pair's finalize is deferred into the next pair (Ln at pair start
  frees the PSUM slot for reuse).  Each chunk's output projection,
  with each head's q-rope staggered one projection behind its PSUM
  copy and the kq transposes after all projections (clear of the DVE
  quant chain), runs as a dense
  software-pipelined PE block at the next chunk boundary -- all 8 PSUM
  banks are free there, the ACT engine gets a breather between
  exp-heavy pairs, and the first block unit rides the attn slot so it
  never waits on the exp backlog.  outT partials are stored bf16 in dt
  pairs (one wide cast alternating DVE/ACT, one store alternating
  sync/gpsimd); the host accumulates the 4 head-group partials in
  fp32.
"""

import numpy as np

import bass_rust
import concourse.bass as bass
import concourse.tile as tile
import concourse.mybir as mybir
from concourse.bass_utils import run_bass_kernel_spmd

B, S, D = 2, 2048, 2048
NH, NKV, HD = 16, 4, 128
GQ = 512            # q dims per core (4 heads)
NKO = D // 128      # 16 contraction tiles
PC = 512            # projection/attention chunk width (tokens)
NPC = S // PC       # 4
QC = 512
NQC = S // QC       # 4
MAGIC = float(np.float32(12582912.0))  # 1.5 * 2**23
SM_SCALE = 1.0 / float(np.sqrt(HD))

F32 = mybir.dt.float32
F32R = mybir.dt.float32r
BF16 = mybir.dt.bfloat16
MULT = mybir.AluOpType.mult
ADD = mybir.AluOpType.add
EXP = mybir.ActivationFunctionType.Exp

_CACHE = {}

# retained after each kernel() call so test harnesses can read profiling info
LAST_RESULTS = None


def _split_multi_waits(nc):
    """This walrus build caps sync waits at 1 per instruction.  Hoist extra
    waits onto single-wait NoOps immediately preceding the instruction on
    the same engine (identical semantics: the engine is in-order)."""
    for f in nc.m.functions:
        for bb in f.blocks:
            new = []
            for inst in bb.instructions:
                si = inst.sync_info
                if si is None:
                    new.append(inst)
                    continue
                waits = list(si.on_wait)
                if len(waits) > 1:
                    for k, w in enumerate(waits[:-1]):
                        nop = mybir.InstNoOp(name=f"{inst.name}-w{k}", ins=[], outs=[])
                        nop.engine = inst.engine
                        nop.sync_info = bass_rust.SyncInfo(on_wait=[w], on_update=[])
                        new.append(nop)
                    inst.sync_info = bass_rust.SyncInfo(
                        on_wait=[waits[-1]], on_update=list(si.on_update)
                    )
                new.append(inst)
            bb.instructions = new


def _host_consts():
    theta = 10000.0
    angles = 1.0 / theta ** (np.arange(0, HD, 2, dtype=np.float32) / HD)
    emb = np.outer(np.arange(S, dtype=np.float32), angles)
    emb = np.concatenate([emb, emb], axis=-1)          # [S, HD]
    cos = np.cos(emb).astype(np.float32)               # [S, HD]
    sin = np.sin(emb).astype(np.float32)
    cosT = np.ascontiguousarray(cos.T)                 # [128, S]
    sinT = np.ascontiguousarray(sin.T)

    # [t, d]-layout tables for k rope: [p, t_tile, hd]
    ctd = np.ascontiguousarray(cos.reshape(S // 128, 128, HD).transpose(1, 0, 2))
    std = sin.reshape(S // 128, 128, HD).transpose(1, 0, 2).copy()
    sgn = std.copy()
    sgn[:, :, : HD // 2] = -std[:, :, : HD // 2]       # sign-folded sin
    sgn = np.ascontiguousarray(sgn)

    rot = np.zeros((128, 128), dtype=np.float32)       # lhsT of rotate_half
    for i in range(64):
        rot[i, i + 64] = 1.0
        rot[i + 64, i] = -1.0

    p = np.arange(128)[:, None]
    f = np.arange(128)[None, :]
    tril = (p <= f).astype(np.float32)                 # key p visible to q f

    ones = np.ones((128, 128), dtype=np.float32)
    ident = np.eye(128, dtype=np.float32)
    import ml_dtypes
    bf16 = ml_dtypes.bfloat16
    return {
        "cosT": cosT.astype(bf16), "sinT": sinT.astype(bf16),
        "ctd": ctd.astype(bf16), "sgn": sgn.astype(bf16),
        "rot": rot.astype(bf16), "tril": tril.astype(bf16),
        "ones": ones.astype(bf16), "ident": ident,
    }


def _build_nc():
    nc = bass.Bass("TRN2", target_bir_lowering=False, debug=False)

    # host pre-arranges dataT/wq/wkv into partition-major layouts so every
    # DMA element is >=4KB contiguous (512B elements run ~3x slower)
    dataT = nc.dram_tensor("dataT", [128, NPC, NKO, PC], BF16,
                           kind="ExternalInput").ap()
    wq = nc.dram_tensor("wq", [128, NKO, GQ], BF16, kind="ExternalInput").ap()
    wkv = nc.dram_tensor("wkv", [128, NKO, 2 * HD], BF16,
                         kind="ExternalInput").ap()
    wo = nc.dram_tensor("wo", [GQ, D], BF16, kind="ExternalInput").ap()
    cosT_d = nc.dram_tensor("cosT", [128, S], BF16, kind="ExternalInput").ap()
    sinT_d = nc.dram_tensor("sinT", [128, S], BF16, kind="ExternalInput").ap()
    ctd_d = nc.dram_tensor("ctd", [128, NKO, HD], BF16, kind="ExternalInput").ap()
    sgn_d = nc.dram_tensor("sgn", [128, NKO, HD], BF16, kind="ExternalInput").ap()
    rot_d = nc.dram_tensor("rot", [128, 128], BF16, kind="ExternalInput").ap()
    tril_d = nc.dram_tensor("tril", [128, 128], BF16, kind="ExternalInput").ap()
    ones_d = nc.dram_tensor("ones", [128, 128], BF16, kind="ExternalInput").ap()
    ident_d = nc.dram_tensor("ident", [128, 128], F32R, kind="ExternalInput").ap()
    outT = nc.dram_tensor("outT", [D, S], BF16, kind="ExternalOutput").ap()

    dataT_r = dataT                                          # [128, 4, 16, PC]
    wq_r = wq                                                # [128, 16, 512]
    wkv_r = wkv                                              # [128, 16, 256]
    wo_r = wo.rearrange("(h p) n -> p h n", p=128)           # [128, 4, S]
    outT_p = outT.rearrange("(dt p) t -> p dt t", p=128)     # [128, 16, S]

    from contextlib import ExitStack
    with tile.TileContext(nc) as tc, ExitStack() as stack:
        small_consts = stack.enter_context(tc.tile_pool(name="sconsts", bufs=1))
        rot_sb = small_consts.tile([128, 128], BF16)
        ones_sb = small_consts.tile([128, 128], BF16)
        id_sb = small_consts.tile([128, 128], F32R)
        tril_sb = small_consts.tile([128, 128], BF16)

        persist = stack.enter_context(tc.tile_pool(name="persist", bufs=1))
        xq4 = persist.tile([128, 4, S], BF16, name="xq4")    # roped q, [d, h, t]
        kt4 = persist.tile([128, 4, QC], BF16, name="kt4")   # quant k, [d, g, t]
        v_g = [persist.tile([128, 4, HD], BF16, tag=f"vg{g}", name=f"v_g{g}")
               for g in range(4)]                            # quant v, [t, j, d]
        wo_t = [persist.tile([128, S], BF16, tag=f"wo{h}", name=f"wo{h}")
                for h in range(4)]                           # loaded mid-phase-1

        GRP = 4

        # ---------------- Phase 1: projections + rope + quant ----------------
        with tc.tile_pool(name="p1consts", bufs=1) as p1c, \
             tc.tile_pool(name="wpool", bufs=1) as wpool, \
             tc.tile_pool(name="datapool", bufs=2) as datapool, \
             tc.tile_pool(name="kvstage", bufs=2) as kvstage, \
             tc.tile_pool(name="qtmp", bufs=2) as qtmp, \
             tc.tile_pool(name="t2pool", bufs=3) as t2pool, \
             tc.tile_pool(name="proj_ps", bufs=3, space="PSUM") as proj_ps, \
             tc.tile_pool(name="kv_ps", bufs=2, space="PSUM") as kv_ps, \
             tc.tile_pool(name="rope_ps", bufs=2, space="PSUM") as rope_ps, \
             tc.tile_pool(name="tp_ps", bufs=1, space="PSUM") as tp_ps:
            cos_sb = p1c.tile([128, S], BF16)
            sin_sb = p1c.tile([128, S], BF16)
            ctd_sb = p1c.tile([128, NKO, HD], BF16)
            sgn_sb = p1c.tile([128, NKO, HD], BF16)
            wq_sb = wpool.tile([128, NKO, GQ], BF16)
            wkv_sb = wpool.tile([128, NKO, 2 * HD], BF16)

            dT = {}
            for c in range(2):
                dT[c] = datapool.tile([128, NKO, PC], BF16, tag="dT",
                                      name=f"dT{c}")

            # initial loads across all 3 DMA-capable queues (sync/gpsimd/
            # scalar).  The DMA arbiter round-robins PACKETS across queues,
            # so a queue carrying small-element descriptors gets starved:
            # big 4-8KB-element transfers go first in each queue's FIFO.
            # The first-needed tensors are QUARTERED so the kv projection's
            # ko loop starts on the first 0.5MB (per-region tile deps) and
            # trickles, instead of waiting for whole halves; ctd/cos only
            # feed DVE chains with slack, so dT1 outranks them.
            nc.sync.dma_start(dT[0][:, 0:4], dataT_r[:, 0, 0:4])
            nc.gpsimd.dma_start(dT[0][:, 8:12], dataT_r[:, 0, 8:12])
            nc.scalar.dma_start(wkv_sb[:, 0:4], wkv_r[:, 0:4])
            nc.sync.dma_start(dT[0][:, 4:8], dataT_r[:, 0, 4:8])
            nc.gpsimd.dma_start(dT[0][:, 12:16], dataT_r[:, 0, 12:16])
            nc.scalar.dma_start(wkv_sb[:, 4:8], wkv_r[:, 4:8])
            nc.scalar.dma_start(wkv_sb[:, 8:12], wkv_r[:, 8:12])
            nc.scalar.dma_start(wkv_sb[:, 12:16], wkv_r[:, 12:16])
            nc.sync.dma_start(wq_sb[:, 0:4], wq_r[:, 0:4])
            nc.sync.dma_start(wq_sb[:, 4:8], wq_r[:, 4:8])
            nc.scalar.dma_start(wq_sb[:, 8:12], wq_r[:, 8:12])
            nc.scalar.dma_start(wq_sb[:, 12:16], wq_r[:, 12:16])
            nc.sync.dma_start(rot_sb[:], rot_d[:])
            nc.sync.dma_start(id_sb[:], ident_d[:])
            nc.gpsimd.dma_start(dT[1][:, 8:16], dataT_r[:, 1, 8:16])
            nc.sync.dma_start(dT[1][:, 0:8], dataT_r[:, 1, 0:8])
            nc.gpsimd.dma_start(ctd_sb[:], ctd_d[:])
            nc.gpsimd.dma_start(cos_sb[:], cosT_d[:])
            nc.scalar.dma_start(sgn_sb[:], sgn_d[:])
            nc.scalar.dma_start(sin_sb[:], sinT_d[:])
            nc.sync.dma_start(ones_sb[:], ones_d[:])
            nc.sync.dma_start(tril_sb[:], tril_d[:])

            # PE warm-up/filler: scratch matmuls during the initial DMA wait
            # keep the HAM clock-gate at K=8/8 so real work runs at full
            # clock, and bridge to the kv projection's first data (~17us in)
            # so no >3.4us idle window re-throttles the clock.
            warm = wpool.tile([128, QC], BF16, name="warm_scratch")
            nc.vector.memset(warm[:], 0.0)

            def warm_fill(n, w=QC):
                for _ in range(n):
                    wps = rope_ps.tile([128, QC], F32, tag="pr")
                    nc.tensor.matmul(wps[:, 0:w], warm[:, 0:128], warm[:, 0:w],
                                     start=True, stop=True)

            # a short warm burst plus a narrow (<=128-col granularity) tail
            # bridges toward the first kv data without delaying it; longer
            # bridges were tried and measured slower -- the DMA ramp and
            # launch-barrier timing vary too much run-to-run to tune the
            # coverage, and overshooting delays real work at full clock
            warm_fill(12)
            warm_fill(16, w=128)

            def quant_group(src_ap, dst_ap):
                amax = qtmp.tile([128, GRP, 1], F32, tag="amax")
                scl = qtmp.tile([128, GRP, 1], F32, tag="scl")
                inv = qtmp.tile([128, GRP, 1], F32, tag="inv")
                xs = qtmp.tile([128, GRP, HD], F32, tag="xs")
                nc.vector.tensor_reduce(amax[:], src_ap, mybir.AxisListType.X,
                                        mybir.AluOpType.max,
                                        apply_absolute_value=True)
                nc.vector.tensor_scalar_max(amax[:], amax[:], 1e-8)
                nc.vector.tensor_scalar_mul(scl[:], amax[:], 1.0 / 127.0)
                nc.vector.reciprocal(inv[:], scl[:])
                sclb = scl[:].to_broadcast((128, GRP, HD))
                invb = inv[:].to_broadcast((128, GRP, HD))
                nc.vector.tensor_tensor(xs[:], src_ap, invb, MULT)
                nc.vector.tensor_scalar_add(xs[:], xs[:], MAGIC)
                nc.vector.tensor_scalar_add(xs[:], xs[:], -MAGIC)
                nc.vector.tensor_tensor(dst_ap, xs[:], sclb, MULT)

            for c in range(NPC):
                csl = bass.ts(c, PC)
                if c + 2 < NPC:
                    cb = c + 2
                    t_ = datapool.tile([128, NKO, PC], BF16, tag="dT",
                                       name=f"dT{cb}")
                    dT[cb] = t_
                    eng = nc.gpsimd if cb % 2 else nc.sync
                    eng.dma_start(t_[:, 0:8], dataT_r[:, cb, 0:8])
                    eng.dma_start(t_[:, 8:16], dataT_r[:, cb, 8:16])
                if c == 2:
                    # wo is first needed by the out-projection block at the
                    # first phase-2 chunk boundary; load it mid-phase-1
                    # while the DMA queues are otherwise idle
                    for h in range(4):
                        eng = nc.sync if h % 2 else nc.scalar
                        eng.dma_start(wo_t[h][:], wo_r[:, h])

                # --- k/v projection straight into [t, d] tiles ---
                kv_td = kvstage.tile([128, GRP, 2 * HD], F32, tag="kvtd",
                                     name=f"kvtd{c}")
                for j in range(GRP):
                    pkv = kv_ps.tile([128, 2 * HD], F32, tag="pkv")
                    for ko in range(NKO):
                        nc.tensor.matmul(pkv[:],
                                         dT[c][:, ko, bass.ds(j * 128, 128)],
                                         wkv_sb[:, ko],
                                         start=(ko == 0), stop=(ko == NKO - 1))
                    nc.scalar.copy(kv_td[:, j, :], pkv[:])

                # --- k rope along free axis (sign-folded sin table) ---
                kr = kvstage.tile([128, GRP, HD], F32, tag="kr", name=f"kr{c}")
                t2k = qtmp.tile([128, GRP, HD], F32, tag="t2k")
                tsl = bass.ts(c, GRP)  # 4 token tiles of this group
                nc.vector.tensor_tensor(kr[:], kv_td[:, :, 0:HD],
                                        ctd_sb[:, tsl], MULT)
                nc.vector.tensor_tensor(t2k[:, :, 0:64],
                                        kv_td[:, :, 64:HD],
                                        sgn_sb[:, tsl, 0:64], MULT)
                nc.vector.tensor_tensor(t2k[:, :, 64:HD],
                                        kv_td[:, :, 0:64],
                                        sgn_sb[:, tsl, 64:HD], MULT)
                nc.vector.tensor_tensor(kr[:], kr[:], t2k[:], ADD)

                # --- int8 quant-dequant (k roped, v raw); v rides here too
                # so the chunk's DVE work finishes early: the last chunk's
                # DVE tail otherwise delays the phase-2 pool handover ---
                kq = kvstage.tile([128, GRP, HD], F32R, tag="kq", name=f"kq{c}")
                quant_group(kr[:], kq[:])
                quant_group(kv_td[:, :, HD:], v_g[c][:])

                # --- q projection per head, with each head's rope staggered
                # one projection behind its copy (so the PE never waits on
                # the ACT copy), and the kq transposes last (the DVE quant
                # chain is guaranteed done by then, and kt4 isn't read until
                # phase 2) ---
                def emit_qproj(h):
                    pq = proj_ps.tile([128, QC], F32, tag="pq",
                                      name=f"pq{c}_{h}")
                    for ko in range(NKO):
                        nc.tensor.matmul(pq[:], wq_sb[:, ko, bass.ts(h, 128)],
                                         dT[c][:, ko],
                                         start=(ko == 0), stop=(ko == NKO - 1))
                    nc.scalar.copy(xq4[:, h, csl], pq[:])

                def emit_rope(h):
                    pr = rope_ps.tile([128, QC], F32, tag="pr")
                    nc.tensor.matmul(pr[:], rot_sb[:], xq4[:, h, csl],
                                     start=True, stop=True)
                    t1 = t2pool.tile([128, QC], BF16, tag="t1")
                    t2 = t2pool.tile([128, QC], BF16, tag="t2")
                    nc.vector.tensor_tensor(t1[:], xq4[:, h, csl],
                                            cos_sb[:, csl], MULT)
                    nc.vector.tensor_tensor(t2[:], pr[:], sin_sb[:, csl], MULT)
                    nc.vector.tensor_tensor(xq4[:, h, csl], t1[:], t2[:], ADD)

                emit_qproj(0)
                emit_qproj(1)
                emit_rope(0)
                emit_qproj(2)
                emit_rope(1)
                emit_qproj(3)
                emit_rope(2)
                for j in range(GRP):
                    pt = tp_ps.tile([128, 128], F32R, tag="tp")
                    nc.tensor.transpose(pt[:], kq[:, j, :], id_sb[:])
                    nc.scalar.copy(kt4[:, c, bass.ts(j, 128)], pt[:])
                emit_rope(3)

        # ---------------- Phase 2: attention + output projection ----------------
        # ki tiles are processed in units of 2 with [128,2,QC] "wide" tiles
        # spanning 2 PSUM banks / 2KB-per-partition SBUF spans: one exp per
        # off-diagonal unit (amortizes the 352-cycle ACT pipeline fill), one
        # staging copy / store per unit.  The softmax denominator rides the
        # PE as per-ki ones-matmuls (engine-side accumulation measured ~2x
        # slower and starves the PE with serial chains).  Each chunk's
        # output projection runs as a dense PE block at the next chunk
        # boundary, when all 8 PSUM banks are free and the ACT engine gets
        # a breather between exp-heavy pairs.
        with tc.tile_pool(name="attn_sb", bufs=5) as attn_sb, \
             tc.tile_pool(name="exp_pool", bufs=7) as exp_pool, \
             tc.tile_pool(name="araw", bufs=3) as araw_pool, \
             tc.tile_pool(name="rc4p", bufs=2) as rc_pool, \
             tc.tile_pool(name="psum_sb", bufs=7) as psum_pool, \
             tc.tile_pool(name="outstage", bufs=4) as outstage, \
             tc.tile_pool(name="score_ps", bufs=2, space="PSUM") as score_ps, \
             tc.tile_pool(name="attn_ps", bufs=1, space="PSUM") as attn_ps, \
             tc.tile_pool(name="pss_ps", bufs=1, space="PSUM") as pss_ps:

            def out_proj_block(c_prev, tiles, fin):
                # chunk-boundary block: run the previous pair's softmax
                # finalize, then the whole [D, QC] output projection of
                # chunk c_prev software-pipelined 3 units deep over the 4
                # wide PSUM slots (all free at a chunk boundary); the h0/h1
                # lead covers the finalize chain before h2/h3 need its
                # at-tiles.  pu0 rides the attn slot (freed by the DVE ar2
                # copy) so the block's first matmuls never wait on the last
                # pair's ACT exp backlog that still holds the score slots
                fin()
                pools = [(attn_ps, "pa2"), (score_ps, "ps2"),
                         (score_ps, "ps2"), (pss_ps, "pss2")]
                pos = {}

                def finishp(pu):
                    po2 = pos.pop(pu)
                    for half in range(2):
                        dt_ = 2 * pu + half
                        for h2 in (2, 3):
                            at2, sti = tiles[h2]
                            nc.tensor.matmul(po2[:, half],
                                             wo_t[h2][:, bass.ts(dt_, 128)],
                                             at2[:, sti],
                                             start=False, stop=(h2 == 3))
                    ot2 = outstage.tile([128, 2, QC], BF16, tag="ot")
                    if pu % 2:
                        nc.vector.tensor_copy(ot2[:], po2[:])
                    else:
                        nc.scalar.copy(ot2[:], po2[:])
                    eng = nc.gpsimd if pu % 2 else nc.sync
                    eng.dma_start(outT_p[:, 2 * pu:2 * pu + 2,
                                         bass.ts(c_prev, QC)], ot2[:])

                for pu in range(NKO // 2):
                    pool, tag = pools[pu % 4]
                    po2 = pool.tile([128, 2, QC], F32, tag=tag)
                    pos[pu] = po2
                    for half in range(2):
                        dt_ = 2 * pu + half
                        for h2 in (0, 1):
                            at2, sti = tiles[h2]
                            nc.tensor.matmul(po2[:, half],
                                             wo_t[h2][:, bass.ts(dt_, 128)],
                                             at2[:, sti],
                                             start=(h2 == 0), stop=False)
                    if pu >= 3:
                        finishp(pu - 3)
                for pu in range(NKO // 2 - 3, NKO // 2):
                    finishp(pu)

            LAG = 2  # units the score/exp pipeline leads the pa matmuls by

            def emit_pair(c, hA, hB, attn_tiles, carry_in):
                nki = 4 * (c + 1)
                U = nki // 2
                streams = (hA, hB)
                if carry_in is not None:
                    # previous pair's Ln runs first so its pss slot frees
                    # before this pair's ones-matmuls need it
                    carry_in[0]()
                pa2 = attn_ps.tile([128, 2, QC], F32, tag="pa2",
                                   name=f"pa2_{c}_{hA}")
                pss2 = pss_ps.tile([128, 2, QC], F32, tag="pss2",
                                   name=f"pss2_{c}_{hA}")

                def emit_acc(u, et2s, qo, psms):
                    diag = qo[1] != 0
                    for st in range(2):
                        if psms[st] is not None:
                            # odd off-diagonal unit: the level-2 pairwise
                            # sum covers THIS unit and the previous one, so
                            # one ones-matmul covers four ki tiles (even
                            # off-diagonal units emit no ones-matmul at all)
                            nc.tensor.matmul(
                                pss2[:, st], ones_sb[:], psms[st][:],
                                start=(u == 1), stop=False)
                        for half in range(2):
                            ki = 2 * u + half
                            q = qo[half]
                            if diag:
                                nc.tensor.matmul(
                                    pss2[:, st, q:], ones_sb[:],
                                    et2s[st][:, half, q:],
                                    start=(ki == 0), stop=(ki == nki - 1))
                            nc.tensor.matmul(
                                pa2[:, st, q:], v_g[ki // 4][:, ki % 4],
                                et2s[st][:, half, q:],
                                start=(ki == 0), stop=(ki == nki - 1))

                pending = []
                last_psm = [None, None]
                for u in range(U):
                    k0 = 2 * u
                    diag = k0 >= 4 * c
                    qo = (128 * (k0 - 4 * c), 128 * (k0 + 1 - 4 * c)) \
                        if diag else (0, 0)
                    et2s = []
                    psms = []
                    q0 = qo[0]
                    for st in range(2):
                        h = streams[st]
                        ps2 = score_ps.tile([128, 2, QC], F32, tag="ps2")
                        for half in range(2):
                            # both halves score from q0: the diagonal
                            # half-1 computes 128 extra (masked, never
                            # read) columns so the unit exps as ONE wide
                            # ACT instruction -- ACT is the co-critical
                            # engine, the extra PE columns are cheap
                            nc.tensor.matmul(
                                ps2[:, half, q0:],
                                kt4[:, (k0 + half) // 4,
                                    bass.ts((k0 + half) % 4, 128)],
                                xq4[:, h, bass.ds(c * QC + q0, QC - q0)],
                                start=True, stop=True)
                        et2 = exp_pool.tile([128, 2, QC], BF16, tag="et2")
                        et2s.append(et2)
                        nc.scalar.activation(et2[:, :, q0:], ps2[:, :, q0:],
                                             EXP, scale=SM_SCALE)
                        if diag:
                            for half in range(2):
                                q = qo[half]
                                nc.gpsimd.tensor_tensor(
                                    et2[:, half, q:q + 128],
                                    et2[:, half, q:q + 128],
                                    tril_sb[:], MULT)
                            psms.append(None)
                        else:
                            # chain-free pairwise sums of the exp tiles
                            # (alternating engines by stream): level 1 sums
                            # the unit's two tiles, level 2 sums consecutive
                            # off-diagonal units, so the PE runs one
                            # ones-matmul per FOUR ki tiles.  Each add has
                            # LAG units of slack before emit_acc reads it.
                            psm = psum_pool.tile([128, QC], BF16, tag="psm")
                            peng = nc.gpsimd if st == 0 else nc.vector
                            peng.tensor_tensor(psm[:], et2[:, 0], et2[:, 1],
                                               ADD)
                            if u % 2 == 0:
                                last_psm[st] = psm
                                psms.append(None)
                            else:
                                psm2 = psum_pool.tile([128, QC], BF16,
                                                      tag="psm")
                                peng.tensor_tensor(psm2[:], last_psm[st][:],
                                                   psm[:], ADD)
                                psms.append(psm2)
                    pending.append((u, et2s, qo, psms))
                    if u >= LAG:
                        emit_acc(*pending.pop(0))
                    if u == 1 and carry_in is not None:
                        carry_in[1]()
                for item in pending:
                    emit_acc(*item)
                # stage the attention accumulator out of PSUM (one wide copy)
                ar2 = araw_pool.tile([128, 2, QC], F32, tag="araw",
                                     name=f"ar2_{c}_{hA}")
                nc.vector.tensor_copy(ar2[:], pa2[:])

                # 1/Z = exp(-ln(Z)) on ACT: Ln and Exp share an ACT function
                # table, so no ACT_TABLE_LOAD ever splits the exp stream,
                # and at ~2.4us the pair is far cheaper than a DVE
                # reciprocal (~4.3us for [128,2,512] -- measured).  fin_a
                # (Ln, reading the PSUM accumulator directly) runs at the
                # next pair's start; fin_b at its second unit.
                state = {}

                def fin_a():
                    lnt = rc_pool.tile([128, 2, QC], F32, tag="lnt")
                    state["lnt"] = lnt
                    nc.scalar.activation(lnt[:], pss2[:],
                                         mybir.ActivationFunctionType.Ln)

                def fin_b():
                    rc2 = rc_pool.tile([128, 2, QC], F32, tag="rc4")
                    nc.scalar.activation(rc2[:], state["lnt"][:], EXP,
                                         scale=-1.0)
                    at2 = attn_sb.tile([128, 2, QC], BF16, tag="attnT")
                    # per-stream multiplies: stream 0's at-tile lands ~0.6us
                    # earlier, unblocking the out-proj block's h2 matmuls
                    for st in range(2):
                        nc.vector.tensor_tensor(at2[:, st], ar2[:, st],
                                                rc2[:, st], MULT)
                        attn_tiles[streams[st]] = (at2, st)

                def fin_tail():
                    fin_a()
                    fin_b()
                return fin_a, fin_b, fin_tail

            # attention chunks are mutually independent by phase-2 start, so
            # the processing order is free: chunk 1 goes first (its pairs
            # are PE-dense enough to hold the HAM clock-gate at full rate,
            # where the tiny ACT-latency-bound chunk 0 measurably
            # re-throttles it), and chunk 0 is sandwiched between two dense
            # out-projection blocks so its PE idle never spans a full HAM
            # window
            prev = None
            for c in (1, 0, 2, 3):
                attn_tiles = {}
                if prev is not None:
                    out_proj_block(prev[0], prev[1], prev[2])
                carry = emit_pair(c, 0, 1, attn_tiles, None)
                carry = emit_pair(c, 2, 3, attn_tiles, carry)
                prev = (c, attn_tiles, carry[2])
            out_proj_block(prev[0], prev[1], prev[2])

    _split_multi_waits(nc)
    return nc


def _get_state():
    if "nc" not in _CACHE:
        _CACHE["nc"] = _build_nc()
        _CACHE["consts"] = _host_consts()
    return _CACHE["nc"], _CACHE["consts"]


def kernel(data=None, mask=None, wq=None, wk=None, wv=None, wo=None, **extra):
    global LAST_RESULTS
    import ml_dtypes
    bf16 = ml_dtypes.bfloat16
    nc, consts = _get_state()

    data = np.asarray(data, dtype=np.float32)
    wq = np.asarray(wq, dtype=np.float32)
    wk = np.asarray(wk, dtype=np.float32)
    wv = np.asarray(wv, dtype=np.float32)
    wo = np.asarray(wo, dtype=np.float32)

    in_maps = []
    # dataT host layout [128, chunk, ko, t]: every DMA element is >=1KB and
    # per-(partition, chunk) spans are 16KB contiguous
    dTs = [np.ascontiguousarray(
        data[b].T.reshape(NKO, 128, NPC, PC).transpose(1, 2, 0, 3)
    ).astype(bf16) for b in range(B)]
    wq_h = [np.ascontiguousarray(
        wq[:, g * GQ:(g + 1) * GQ].reshape(NKO, 128, GQ).transpose(1, 0, 2)
    ).astype(bf16) for g in range(NKV)]
    wkv_h = [np.ascontiguousarray(
        np.concatenate([wk[:, g * HD:(g + 1) * HD],
                        wv[:, g * HD:(g + 1) * HD]], axis=1)
        .reshape(NKO, 128, 2 * HD).transpose(1, 0, 2)
    ).astype(bf16) for g in range(NKV)]
    for b in range(B):
        for g in range(NKV):
            in_maps.append({
                "dataT": dTs[b],
                "wq": wq_h[g],
                "wkv": wkv_h[g],
                "wo": np.ascontiguousarray(wo[g * GQ:(g + 1) * GQ, :]).astype(bf16),
                "cosT": consts["cosT"],
                "sinT": consts["sinT"],
                "ctd": consts["ctd"],
                "sgn": consts["sgn"],
                "rot": consts["rot"],
                "tril": consts["tril"],
                "ones": consts["ones"],
                "ident": consts["ident"],
            })

    res = run_bass_kernel_spmd(nc, in_maps, core_ids=list(range(8)))
    LAST_RESULTS = res

    out = np.empty((B, S, D), dtype=np.float32)
    for b in range(B):
        acc = res.results[b * NKV]["outT"].astype(np.float32).copy()
        for g in range(1, NKV):
            acc += res.results[b * NKV + g]["outT"]
        out[b] = acc.T
    return out



# revision 65
# speedup vs baseline: 1.0681x; 1.0044x over previous
"""Trainium2 Bass kernel for nn_Attention_197568495719.

Full attention layer: QKV projection + RoPE + int8 KV quant-dequant + GQA
causal SDPA + output projection.  B=2, S=2048, D=2048, 16 q heads / 4 kv
heads, head_dim=128.

Sharding: 8 cores = 2 (batch) x 4 (kv-head groups).  Core (b, g) computes
batch b with q heads 4g..4g+3 and kv head g (tensor parallel on heads:
wq/wk/wv split on output dim, wo on input dim).  Each core produces a
partial outT = (attn @ wo_g).T in [D, S] layout; the host sums the 4
group partials per batch and transposes back.

Design (v3, ~287-304us measured depending on the device's bimodal
clock mode; v2 was ~288-342us, v1 ~485us):
- Everything on the PE is bf16 (hardware fp32r "HIGH" mode multiplies
  with bf16-truncated operands anyway, but pays a ~70ns un-hidden
  fp32 LDWEIGHTS per matmul since FWL is fp32-disabled -- bf16 is
  numerically equivalent and strictly faster).  Host pre-arranges
  dataT/wq/wkv partition-major so every DMA element is >=4KB (512B
  elements run ~3x slower, and small-element descriptors starve their
  whole queue at the packet-round-robin arbiter -- tiny consts ride at
  queue tails).  Initial loads fan out over all three DMA-capable
  queues (sync/gpsimd/scalar) ordered by first use; wo loads ride the
  idle mid-phase-1 DMA window.  A bf16 scratch-matmul burst (512- then
  128-col) bridges the DMA-bound head so the HAM clock-gate never
  re-throttles before real work arrives.
- Phase 1 (projections): 512-token chunks; k/v projected directly into
  [token, dim] tiles so the int8 quant path needs no PE transposes in;
  k RoPE runs along the free axis with a sign-folded sin table; q RoPE
  in place per (head, chunk) with rot matmuls deferred behind all four
  head projections.  Quant rounding uses the fp32 +-1.5*2^23 magic-add
  (exact round-half-to-even, matching jnp.round).
- Phase 2 (attention): ki tiles processed in units of 2 with
  [128,2,QC] wide tiles spanning 2 PSUM banks: one ACT exp per unit
  (amortizing the 352-cycle ACT pipeline fill; ACT is the co-critical
  engine -- diagonal units score 128 extra masked-never-read columns
  on their second half so the whole unit exps in one instruction), one
  wide DVE staging copy per pair, wide finalize ops.  Scores race
  LAG=2 units ahead of the accumulating matmuls.  Causal masking
  multiplies only the 128x128 triangular block per diagonal tile on
  GPSIMD; the accumulating matmuls are trimmed to the exact live
  q-range (128j).  The softmax denominator rides the
  PE as ones-matmuls; off-diagonal units' two exp tiles are pre-summed
  element-wise off the PE (chain-free, alternating GPSIMD/DVE by
  stream) so one ones-matmul covers both ki tiles.  (Fully chained
  engine-side accumulation and DVE reciprocal were both tried and
  measured slower: the chains starve the PE, and DVE reciprocal costs
  ~4.3us per [128,2,512].)  1/Z = exp(-ln(Z)) on ACT: Ln/Exp share an
  ACT function table so no ACT_TABLE_LOAD splits the exp stream; each
  pair's finalize is deferred into the next pair (Ln at pair start
  frees the PSUM slot for reuse).  Each chunk's output projection,
  with each head's q-rope staggered one projection behind its PSUM
  copy and the kq transposes after all projections (clear of the DVE
  quant chain), runs as a dense
  software-pipelined PE block at the next chunk boundary -- all 8 PSUM
  banks are free there, the ACT engine gets a breather between
  exp-heavy pairs, and the first block unit rides the attn slot so it
  never waits on the exp backlog.  outT partials are stored bf16 in dt
  pairs (one wide cast alternating DVE/ACT, one store alternating
  sync/gpsimd); the host accumulates the 4 head-group partials in
  fp32.
"""

import numpy as np

import bass_rust
import concourse.bass as bass
import concourse.tile as tile
import concourse.mybir as mybir
from concourse.bass_utils import run_bass_kernel_spmd

B, S, D = 2, 2048, 2048
NH, NKV, HD = 16, 4, 128
GQ = 512            # q dims per core (4 heads)
NKO = D // 128      # 16 contraction tiles
PC = 512            # projection/attention chunk width (tokens)
NPC = S // PC       # 4
QC = 512
NQC = S // QC       # 4
MAGIC = float(np.float32(12582912.0))  # 1.5 * 2**23
SM_SCALE = 1.0 / float(np.sqrt(HD))

F32 = mybir.dt.float32
F32R = mybir.dt.float32r
BF16 = mybir.dt.bfloat16
MULT = mybir.AluOpType.mult
ADD = mybir.AluOpType.add
EXP = mybir.ActivationFunctionType.Exp

_CACHE = {}

# retained after each kernel() call so test harnesses can read profiling info
LAST_RESULTS = None


def _split_multi_waits(nc):
    """This walrus build caps sync waits at 1 per instruction.  Hoist extra
    waits onto single-wait NoOps immediately preceding the instruction on
    the same engine (identical semantics: the engine is in-order)."""
    for f in nc.m.functions:
        for bb in f.blocks:
            new = []
            for inst in bb.instructions:
                si = inst.sync_info
                if si is None:
                    new.append(inst)
                    continue
                waits = list(si.on_wait)
                if len(waits) > 1:
                    for k, w in enumerate(waits[:-1]):
                        nop = mybir.InstNoOp(name=f"{inst.name}-w{k}", ins=[], outs=[])
                        nop.engine = inst.engine
                        nop.sync_info = bass_rust.SyncInfo(on_wait=[w], on_update=[])
                        new.append(nop)
                    inst.sync_info = bass_rust.SyncInfo(
                        on_wait=[waits[-1]], on_update=list(si.on_update)
                    )
                new.append(inst)
            bb.instructions = new


def _host_consts():
    theta = 10000.0
    angles = 1.0 / theta ** (np.arange(0, HD, 2, dtype=np.float32) / HD)
    emb = np.outer(np.arange(S, dtype=np.float32), angles)
    emb = np.concatenate([emb, emb], axis=-1)          # [S, HD]
    cos = np.cos(emb).astype(np.float32)               # [S, HD]
    sin = np.sin(emb).astype(np.float32)
    cosT = np.ascontiguousarray(cos.T)                 # [128, S]
    sinT = np.ascontiguousarray(sin.T)

    # [t, d]-layout tables for k rope: [p, t_tile, hd]
    ctd = np.ascontiguousarray(cos.reshape(S // 128, 128, HD).transpose(1, 0, 2))
    std = sin.reshape(S // 128, 128, HD).transpose(1, 0, 2).copy()
    sgn = std.copy()
    sgn[:, :, : HD // 2] = -std[:, :, : HD // 2]       # sign-folded sin
    sgn = np.ascontiguousarray(sgn)

    rot = np.zeros((128, 128), dtype=np.float32)       # lhsT of rotate_half
    for i in range(64):
        rot[i, i + 64] = 1.0
        rot[i + 64, i] = -1.0

    p = np.arange(128)[:, None]
    f = np.arange(128)[None, :]
    tril = (p <= f).astype(np.float32)                 # key p visible to q f

    ones = np.ones((128, 128), dtype=np.float32)
    ident = np.eye(128, dtype=np.float32)
    import ml_dtypes
    bf16 = ml_dtypes.bfloat16
    return {
        "cosT": cosT.astype(bf16), "sinT": sinT.astype(bf16),
        "ctd": ctd.astype(bf16), "sgn": sgn.astype(bf16),
        "rot": rot.astype(bf16), "tril": tril.astype(bf16),
        "ones": ones.astype(bf16), "ident": ident,
    }


def _build_nc():
    nc = bass.Bass("TRN2", target_bir_lowering=False, debug=False)

    # host pre-arranges dataT/wq/wkv into partition-major layouts so every
    # DMA element is >=4KB contiguous (512B elements run ~3x slower)
    dataT = nc.dram_tensor("dataT", [128, NPC, NKO, PC], BF16,
                           kind="ExternalInput").ap()
    wq = nc.dram_tensor("wq", [128, NKO, GQ], BF16, kind="ExternalInput").ap()
    wkv = nc.dram_tensor("wkv", [128, NKO, 2 * HD], BF16,
                         kind="ExternalInput").ap()
    wo = nc.dram_tensor("wo", [GQ, D], BF16, kind="ExternalInput").ap()
    cosT_d = nc.dram_tensor("cosT", [128, S], BF16, kind="ExternalInput").ap()
    sinT_d = nc.dram_tensor("sinT", [128, S], BF16, kind="ExternalInput").ap()
    ctd_d = nc.dram_tensor("ctd", [128, NKO, HD], BF16, kind="ExternalInput").ap()
    sgn_d = nc.dram_tensor("sgn", [128, NKO, HD], BF16, kind="ExternalInput").ap()
    rot_d = nc.dram_tensor("rot", [128, 128], BF16, kind="ExternalInput").ap()
    tril_d = nc.dram_tensor("tril", [128, 128], BF16, kind="ExternalInput").ap()
    ones_d = nc.dram_tensor("ones", [128, 128], BF16, kind="ExternalInput").ap()
    ident_d = nc.dram_tensor("ident", [128, 128], F32R, kind="ExternalInput").ap()
    outT = nc.dram_tensor("outT", [D, S], BF16, kind="ExternalOutput").ap()

    dataT_r = dataT                                          # [128, 4, 16, PC]
    wq_r = wq                                                # [128, 16, 512]
    wkv_r = wkv                                              # [128, 16, 256]
    wo_r = wo.rearrange("(h p) n -> p h n", p=128)           # [128, 4, S]
    outT_p = outT.rearrange("(dt p) t -> p dt t", p=128)     # [128, 16, S]

    from contextlib import ExitStack
    with tile.TileContext(nc) as tc, ExitStack() as stack:
        small_consts = stack.enter_context(tc.tile_pool(name="sconsts", bufs=1))
        rot_sb = small_consts.tile([128, 128], BF16)
        ones_sb = small_consts.tile([128, 128], BF16)
        id_sb = small_consts.tile([128, 128], F32R)
        tril_sb = small_consts.tile([128, 128], BF16)

        persist = stack.enter_context(tc.tile_pool(name="persist", bufs=1))
        xq4 = persist.tile([128, 4, S], BF16, name="xq4")    # roped q, [d, h, t]
        kt4 = persist.tile([128, 4, QC], BF16, name="kt4")   # quant k, [d, g, t]
        v_g = [persist.tile([128, 4, HD], BF16, tag=f"vg{g}", name=f"v_g{g}")
               for g in range(4)]                            # quant v, [t, j, d]
        wo_t = [persist.tile([128, S], BF16, tag=f"wo{h}", name=f"wo{h}")
                for h in range(4)]                           # loaded mid-phase-1

        GRP = 4

        # ---------------- Phase 1: projections + rope + quant ----------------
        with tc.tile_pool(name="p1consts", bufs=1) as p1c, \
             tc.tile_pool(name="wpool", bufs=1) as wpool, \
             tc.tile_pool(name="datapool", bufs=2) as datapool, \
             tc.tile_pool(name="kvstage", bufs=2) as kvstage, \
             tc.tile_pool(name="qtmp", bufs=2) as qtmp, \
             tc.tile_pool(name="t2pool", bufs=3) as t2pool, \
             tc.tile_pool(name="proj_ps", bufs=3, space="PSUM") as proj_ps, \
             tc.tile_pool(name="kv_ps", bufs=2, space="PSUM") as kv_ps, \
             tc.tile_pool(name="rope_ps", bufs=2, space="PSUM") as rope_ps, \
             tc.tile_pool(name="tp_ps", bufs=1, space="PSUM") as tp_ps:
            cos_sb = p1c.tile([128, S], BF16)
            sin_sb = p1c.tile([128, S], BF16)
            ctd_sb = p1c.tile([128, NKO, HD], BF16)
            sgn_sb = p1c.tile([128, NKO, HD], BF16)
            wq_sb = wpool.tile([128, NKO, GQ], BF16)
            wkv_sb = wpool.tile([128, NKO, 2 * HD], BF16)

            dT = {}
            for c in range(2):
                dT[c] = datapool.tile([128, NKO, PC], BF16, tag="dT",
                                      name=f"dT{c}")

            # initial loads across all 3 DMA-capable queues (sync/gpsimd/
            # scalar).  The DMA arbiter round-robins PACKETS across queues,
            # so a queue carrying small-element descriptors gets starved:
            # big 4-8KB-element transfers go first in each queue's FIFO.
            # The first-needed tensors are QUARTERED so the kv projection's
            # ko loop starts on the first 0.5MB (per-region tile deps) and
            # trickles, instead of waiting for whole halves; ctd/cos only
            # feed DVE chains with slack, so dT1 outranks them.
            nc.sync.dma_start(dT[0][:, 0:4], dataT_r[:, 0, 0:4])
            nc.gpsimd.dma_start(dT[0][:, 8:12], dataT_r[:, 0, 8:12])
            nc.scalar.dma_start(wkv_sb[:, 0:4], wkv_r[:, 0:4])
            nc.sync.dma_start(dT[0][:, 4:8], dataT_r[:, 0, 4:8])
            nc.gpsimd.dma_start(dT[0][:, 12:16], dataT_r[:, 0, 12:16])
            nc.scalar.dma_start(wkv_sb[:, 4:8], wkv_r[:, 4:8])
            nc.scalar.dma_start(wkv_sb[:, 8:12], wkv_r[:, 8:12])
            nc.scalar.dma_start(wkv_sb[:, 12:16], wkv_r[:, 12:16])
            nc.sync.dma_start(wq_sb[:, 0:4], wq_r[:, 0:4])
            nc.sync.dma_start(wq_sb[:, 4:8], wq_r[:, 4:8])
            nc.scalar.dma_start(wq_sb[:, 8:12], wq_r[:, 8:12])
            nc.scalar.dma_start(wq_sb[:, 12:16], wq_r[:, 12:16])
            nc.sync.dma_start(rot_sb[:], rot_d[:])
            nc.sync.dma_start(id_sb[:], ident_d[:])
            nc.gpsimd.dma_start(dT[1][:, 8:16], dataT_r[:, 1, 8:16])
            nc.sync.dma_start(dT[1][:, 0:8], dataT_r[:, 1, 0:8])
            nc.gpsimd.dma_start(ctd_sb[:], ctd_d[:])
            nc.gpsimd.dma_start(cos_sb[:], cosT_d[:])
            nc.scalar.dma_start(sgn_sb[:], sgn_d[:])
            nc.scalar.dma_start(sin_sb[:], sinT_d[:])
            nc.sync.dma_start(ones_sb[:], ones_d[:])
            nc.sync.dma_start(tril_sb[:], tril_d[:])

            # PE warm-up/filler: scratch matmuls during the initial DMA wait
            # keep the HAM clock-gate at K=8/8 so real work runs at full
            # clock, and bridge to the kv projection's first data (~17us in)
            # so no >3.4us idle window re-throttles the clock.
            warm = wpool.tile([128, QC], BF16, name="warm_scratch")
            nc.vector.memset(warm[:], 0.0)

            def warm_fill(n, w=QC):
                for _ in range(n):
                    wps = rope_ps.tile([128, QC], F32, tag="pr")
                    nc.tensor.matmul(wps[:, 0:w], warm[:, 0:128], warm[:, 0:w],
                                     start=True, stop=True)

            # a short warm burst plus a narrow (<=128-col granularity) tail
            # bridges toward the first kv data without delaying it; longer
            # bridges were tried and measured slower -- the DMA ramp and
            # launch-barrier timing vary too much run-to-run to tune the
            # coverage, and overshooting delays real work at full clock
            warm_fill(12)
            warm_fill(16, w=128)

            def quant_group(src_ap, dst_ap):
                amax = qtmp.tile([128, GRP, 1], F32, tag="amax")
                scl = qtmp.tile([128, GRP, 1], F32, tag="scl")
                inv = qtmp.tile([128, GRP, 1], F32, tag="inv")
                xs = qtmp.tile([128, GRP, HD], F32, tag="xs")
                nc.vector.tensor_reduce(amax[:], src_ap, mybir.AxisListType.X,
                                        mybir.AluOpType.max,
                                        apply_absolute_value=True)
                nc.vector.tensor_scalar_max(amax[:], amax[:], 1e-8)
                nc.vector.tensor_scalar_mul(scl[:], amax[:], 1.0 / 127.0)
                nc.vector.reciprocal(inv[:], scl[:])
                sclb = scl[:].to_broadcast((128, GRP, HD))
                invb = inv[:].to_broadcast((128, GRP, HD))
                nc.vector.tensor_tensor(xs[:], src_ap, invb, MULT)
                nc.vector.tensor_scalar_add(xs[:], xs[:], MAGIC)
                nc.vector.tensor_scalar_add(xs[:], xs[:], -MAGIC)
                nc.vector.tensor_tensor(dst_ap, xs[:], sclb, MULT)

            for c in range(NPC):
                csl = bass.ts(c, PC)
                if c + 2 < NPC:
                    cb = c + 2
                    t_ = datapool.tile([128, NKO, PC], BF16, tag="dT",
                                       name=f"dT{cb}")
                    dT[cb] = t_
                    eng = nc.gpsimd if cb % 2 else nc.sync
                    eng.dma_start(t_[:, 0:8], dataT_r[:, cb, 0:8])
                    eng.dma_start(t_[:, 8:16], dataT_r[:, cb, 8:16])
                if c == 2:
                    # wo is first needed by the out-projection block at the
                    # first phase-2 chunk boundary; load it mid-phase-1
                    # while the DMA queues are otherwise idle
                    for h in range(4):
                        eng = nc.sync if h % 2 else nc.scalar
                        eng.dma_start(wo_t[h][:], wo_r[:, h])

                # --- k/v projection straight into [t, d] tiles ---
                kv_td = kvstage.tile([128, GRP, 2 * HD], F32, tag="kvtd",
                                     name=f"kvtd{c}")
                for j in range(GRP):
                    pkv = kv_ps.tile([128, 2 * HD], F32, tag="pkv")
                    for ko in range(NKO):
                        nc.tensor.matmul(pkv[:],
                                         dT[c][:, ko, bass.ds(j * 128, 128)],
                                         wkv_sb[:, ko],
                                         start=(ko == 0), stop=(ko == NKO - 1))
                    nc.scalar.copy(kv_td[:, j, :], pkv[:])

                # --- k rope along free axis (sign-folded sin table) ---
                kr = kvstage.tile([128, GRP, HD], F32, tag="kr", name=f"kr{c}")
                t2k = qtmp.tile([128, GRP, HD], F32, tag="t2k")
                tsl = bass.ts(c, GRP)  # 4 token tiles of this group
                nc.vector.tensor_tensor(kr[:], kv_td[:, :, 0:HD],
                                        ctd_sb[:, tsl], MULT)
                nc.vector.tensor_tensor(t2k[:, :, 0:64],
                                        kv_td[:, :, 64:HD],
                                        sgn_sb[:, tsl, 0:64], MULT)
                nc.vector.tensor_tensor(t2k[:, :, 64:HD],
                                        kv_td[:, :, 0:64],
                                        sgn_sb[:, tsl, 64:HD], MULT)
                nc.vector.tensor_tensor(kr[:], kr[:], t2k[:], ADD)

                # --- int8 quant-dequant (k roped, v raw); v rides here too
                # so the chunk's DVE work finishes early: the last chunk's
                # DVE tail otherwise delays the phase-2 pool handover ---
                kq = kvstage.tile([128, GRP, HD], F32R, tag="kq", name=f"kq{c}")
                quant_group(kr[:], kq[:])
                quant_group(kv_td[:, :, HD:], v_g[c][:])

                # --- q projection per head, with each head's rope staggered
                # one projection behind its copy (so the PE never waits on
                # the ACT copy), and the kq transposes last (the DVE quant
                # chain is guaranteed done by then, and kt4 isn't read until
                # phase 2) ---
                def emit_qproj(h):
                    pq = proj_ps.tile([128, QC], F32, tag="pq",
                                      name=f"pq{c}_{h}")
                    for ko in range(NKO):
                        nc.tensor.matmul(pq[:], wq_sb[:, ko, bass.ts(h, 128)],
                                         dT[c][:, ko],
                                         start=(ko == 0), stop=(ko == NKO - 1))
                    nc.scalar.copy(xq4[:, h, csl], pq[:])

                def emit_rope(h):
                    pr = rope_ps.tile([128, QC], F32, tag="pr")
                    nc.tensor.matmul(pr[:], rot_sb[:], xq4[:, h, csl],
                                     start=True, stop=True)
                    t1 = t2pool.tile([128, QC], BF16, tag="t1")
                    t2 = t2pool.tile([128, QC], BF16, tag="t2")
                    nc.vector.tensor_tensor(t1[:], xq4[:, h, csl],
                                            cos_sb[:, csl], MULT)
                    nc.vector.tensor_tensor(t2[:], pr[:], sin_sb[:, csl], MULT)
                    nc.vector.tensor_tensor(xq4[:, h, csl], t1[:], t2[:], ADD)

                emit_qproj(0)
                emit_qproj(1)
                emit_rope(0)
                emit_qproj(2)
                emit_rope(1)
                emit_qproj(3)
                emit_rope(2)
                for j in range(GRP):
                    pt = tp_ps.tile([128, 128], F32R, tag="tp")
                    nc.tensor.transpose(pt[:], kq[:, j, :], id_sb[:])
                    nc.scalar.copy(kt4[:, c, bass.ts(j, 128)], pt[:])
                emit_rope(3)

        # ---------------- Phase 2: attention + output projection ----------------
        # ki tiles are processed in units of 2 with [128,2,QC] "wide" tiles
        # spanning 2 PSUM banks / 2KB-per-partition SBUF spans: one exp per
        # off-diagonal unit (amortizes the 352-cycle ACT pipeline fill), one
        # staging copy / store per unit.  The softmax denominator rides the
        # PE as per-ki ones-matmuls (engine-side accumulation measured ~2x
        # slower and starves the PE with serial chains).  Each chunk's
        # output projection runs as a dense PE block at the next chunk
        # boundary, when all 8 PSUM banks are free and the ACT engine gets
        # a breather between exp-heavy pairs.
        with tc.tile_pool(name="attn_sb", bufs=5) as attn_sb, \
             tc.tile_pool(name="exp_pool", bufs=7) as exp_pool, \
             tc.tile_pool(name="araw", bufs=3) as araw_pool, \
             tc.tile_pool(name="rc4p", bufs=2) as rc_pool, \
             tc.tile_pool(name="psum_sb", bufs=7) as psum_pool, \
             tc.tile_pool(name="outstage", bufs=4) as outstage, \
             tc.tile_pool(name="score_ps", bufs=2, space="PSUM") as score_ps, \
             tc.tile_pool(name="attn_ps", bufs=1, space="PSUM") as attn_ps, \
             tc.tile_pool(name="pss_ps", bufs=1, space="PSUM") as pss_ps:

            def out_proj_block(c_prev, tiles, fin):
                # chunk-boundary block: run the previous pair's softmax
                # finalize, then the whole [D, QC] output projection of
                # chunk c_prev software-pipelined 3 units deep over the 4
                # wide PSUM slots (all free at a chunk boundary); the h0/h1
                # lead covers the finalize chain before h2/h3 need its
                # at-tiles.  pu0 rides the attn slot (freed by the DVE ar2
                # copy) so the block's first matmuls never wait on the last
                # pair's ACT exp backlog that still holds the score slots
                fin()
                pools = [(attn_ps, "pa2"), (score_ps, "ps2"),
                         (score_ps, "ps2"), (pss_ps, "pss2")]
                pos = {}

                def finishp(pu):
                    po2 = pos.pop(pu)
                    for half in range(2):
                        dt_ = 2 * pu + half
                        for h2 in (2, 3):
                            at2, sti = tiles[h2]
                            nc.tensor.matmul(po2[:, half],
                                             wo_t[h2][:, bass.ts(dt_, 128)],
                                             at2[:, sti],
                                             start=False, stop=(h2 == 3))
                    ot2 = outstage.tile([128, 2, QC], BF16, tag="ot")
                    if pu % 2:
                        nc.vector.tensor_copy(ot2[:], po2[:])
                    else:
                        nc.scalar.copy(ot2[:], po2[:])
                    eng = nc.gpsimd if pu % 2 else nc.sync
                    eng.dma_start(outT_p[:, 2 * pu:2 * pu + 2,
                                         bass.ts(c_prev, QC)], ot2[:])

                for pu in range(NKO // 2):
                    pool, tag = pools[pu % 4]
                    po2 = pool.tile([128, 2, QC], F32, tag=tag)
                    pos[pu] = po2
                    for half in range(2):
                        dt_ = 2 * pu + half
                        for h2 in (0, 1):
                            at2, sti = tiles[h2]
                            nc.tensor.matmul(po2[:, half],
                                             wo_t[h2][:, bass.ts(dt_, 128)],
                                             at2[:, sti],
                                             start=(h2 == 0), stop=False)
                    if pu >= 3:
                        finishp(pu - 3)
                for pu in range(NKO // 2 - 3, NKO // 2):
                    finishp(pu)

            LAG = 2  # units the score/exp pipeline leads the pa matmuls by

            def emit_pair(c, hA, hB, attn_tiles, carry_in):
                nki = 4 * (c + 1)
                U = nki // 2
                streams = (hA, hB)
                if carry_in is not None:
                    # previous pair's Ln runs first so its pss slot frees
                    # before this pair's ones-matmuls need it
                    carry_in[0]()
                pa2 = attn_ps.tile([128, 2, QC], F32, tag="pa2",
                                   name=f"pa2_{c}_{hA}")
                pss2 = pss_ps.tile([128, 2, QC], F32, tag="pss2",
                                   name=f"pss2_{c}_{hA}")

                def emit_acc(u, et2s, qo, psms):
                    diag = qo[1] != 0
                    for st in range(2):
                        if psms[st] is not None:
                            # odd off-diagonal unit: the level-2 pairwise
                            # sum covers THIS unit and the previous one, so
                            # one ones-matmul covers four ki tiles (even
                            # off-diagonal units emit no ones-matmul at all)
                            nc.tensor.matmul(
                                pss2[:, st], ones_sb[:], psms[st][:],
                                start=(u == 1), stop=False)
                        for half in range(2):
                            ki = 2 * u + half
                            q = qo[half]
                            if diag:
                                nc.tensor.matmul(
                                    pss2[:, st, q:], ones_sb[:],
                                    et2s[st][:, half, q:],
                                    start=(ki == 0), stop=(ki == nki - 1))
                            nc.tensor.matmul(
                                pa2[:, st, q:], v_g[ki // 4][:, ki % 4],
                                et2s[st][:, half, q:],
                                start=(ki == 0), stop=(ki == nki - 1))

                pending = []
                last_psm = [None, None]
                for u in range(U):
                    k0 = 2 * u
                    diag = k0 >= 4 * c
                    qo = (128 * (k0 - 4 * c), 128 * (k0 + 1 - 4 * c)) \
                        if diag else (0, 0)
                    et2s = []
                    psms = []
                    q0 = qo[0]
                    for st in range(2):
                        h = streams[st]
                        ps2 = score_ps.tile([128, 2, QC], F32, tag="ps2")
                        for half in range(2):
                            # both halves score from q0: the diagonal
                            # half-1 computes 128 extra (masked, never
                            # read) columns so the unit exps as ONE wide
                            # ACT instruction -- ACT is the co-critical
                            # engine, the extra PE columns are cheap
                            nc.tensor.matmul(
                                ps2[:, half, q0:],
                                kt4[:, (k0 + half) // 4,
                                    bass.ts((k0 + half) % 4, 128)],
                                xq4[:, h, bass.ds(c * QC + q0, QC - q0)],
                                start=True, stop=True)
                        et2 = exp_pool.tile([128, 2, QC], BF16, tag="et2")
                        et2s.append(et2)
                        nc.scalar.activation(et2[:, :, q0:], ps2[:, :, q0:],
                                             EXP, scale=SM_SCALE)
                        if diag:
                            for half in range(2):
                                q = qo[half]
                                nc.gpsimd.tensor_tensor(
                                    et2[:, half, q:q + 128],
                                    et2[:, half, q:q + 128],
                                    tril_sb[:], MULT)
                            psms.append(None)
                        else:
                            # chain-free pairwise sums of the exp tiles
                            # (alternating engines by stream): level 1 sums
                            # the unit's two tiles, level 2 sums consecutive
                            # off-diagonal units, so the PE runs one
                            # ones-matmul per FOUR ki tiles.  Each add has
                            # LAG units of slack before emit_acc reads it.
                            psm = psum_pool.tile([128, QC], BF16, tag="psm")
                            peng = nc.gpsimd if st == 0 else nc.vector
                            peng.tensor_tensor(psm[:], et2[:, 0], et2[:, 1],
                                               ADD)
                            if u % 2 == 0:
                                last_psm[st] = psm
                                psms.append(None)
                            else:
                                psm2 = psum_pool.tile([128, QC], BF16,
                                                      tag="psm")
                                peng.tensor_tensor(psm2[:], last_psm[st][:],
                                                   psm[:], ADD)
                                psms.append(psm2)
                    pending.append((u, et2s, qo, psms))
                    if u >= LAG:
                        emit_acc(*pending.pop(0))
                    if u == 1 and carry_in is not None:
                        carry_in[1]()
                for item in pending:
                    emit_acc(*item)
                # stage the attention accumulator out of PSUM (one wide DVE
                # copy; splitting it across gpsimd+vector to free the attn
                # slot earlier fails at runtime -- gpsimd can't do this
                # PSUM half-tile copy)
                ar2 = araw_pool.tile([128, 2, QC], F32, tag="araw",
                                     name=f"ar2_{c}_{hA}")
                nc.vector.tensor_copy(ar2[:], pa2[:])

                # 1/Z = exp(-ln(Z)) on ACT: Ln and Exp share an ACT function
                # table, so no ACT_TABLE_LOAD ever splits the exp stream,
                # and at ~2.4us the pair is far cheaper than a DVE
                # reciprocal (~4.3us for [128,2,512] -- measured).  fin_a
                # (Ln, reading the PSUM accumulator directly) runs at the
                # next pair's start; fin_b at its second unit.
                state = {}

                def fin_a():
                    lnt = rc_pool.tile([128, 2, QC], F32, tag="lnt")
                    state["lnt"] = lnt
                    nc.scalar.activation(lnt[:], pss2[:],
                                         mybir.ActivationFunctionType.Ln)

                def fin_b():
                    rc2 = rc_pool.tile([128, 2, QC], F32, tag="rc4")
                    nc.scalar.activation(rc2[:], state["lnt"][:], EXP,
                                         scale=-1.0)
                    at2 = attn_sb.tile([128, 2, QC], BF16, tag="attnT")
                    # per-stream multiplies: stream 0's at-tile lands ~0.6us
                    # earlier, unblocking the out-proj block's h2 matmuls
                    for st in range(2):
                        nc.vector.tensor_tensor(at2[:, st], ar2[:, st],
                                                rc2[:, st], MULT)
                        attn_tiles[streams[st]] = (at2, st)

                def fin_tail():
                    # block-boundary form: per-stream Ln/Exp/mult chains so
                    # stream 0's at-tiles land a chain-length earlier --
                    # the out-proj block's h2 matmuls (stream 0) stop
                    # waiting past its 3-unit h0/h1 lead
                    lnt = rc_pool.tile([128, 2, QC], F32, tag="lnt")
                    rc2 = rc_pool.tile([128, 2, QC], F32, tag="rc4")
                    at2 = attn_sb.tile([128, 2, QC], BF16, tag="attnT")
                    for st in range(2):
                        nc.scalar.activation(lnt[:, st], pss2[:, st],
                                             mybir.ActivationFunctionType.Ln)
                        nc.scalar.activation(rc2[:, st], lnt[:, st], EXP,
                                             scale=-1.0)
                        nc.vector.tensor_tensor(at2[:, st], ar2[:, st],
                                                rc2[:, st], MULT)
                        attn_tiles[streams[st]] = (at2, st)
                return fin_a, fin_b, fin_tail

            # attention chunks are mutually independent by phase-2 start, so
            # the processing order is free: chunk 1 goes first (its pairs
            # are PE-dense enough to hold the HAM clock-gate at full rate,
            # where the tiny ACT-latency-bound chunk 0 measurably
            # re-throttles it), and chunk 0 is sandwiched between two dense
            # out-projection blocks so its PE idle never spans a full HAM
            # window
            prev = None
            for c in (1, 0, 2, 3):
                attn_tiles = {}
                if prev is not None:
                    out_proj_block(prev[0], prev[1], prev[2])
                carry = emit_pair(c, 0, 1, attn_tiles, None)
                carry = emit_pair(c, 2, 3, attn_tiles, carry)
                prev = (c, attn_tiles, carry[2])
            out_proj_block(prev[0], prev[1], prev[2])

    _split_multi_waits(nc)
    return nc


def _get_state():
    if "nc" not in _CACHE:
        _CACHE["nc"] = _build_nc()
        _CACHE["consts"] = _host_consts()
    return _CACHE["nc"], _CACHE["consts"]


def kernel(data=None, mask=None, wq=None, wk=None, wv=None, wo=None, **extra):
    global LAST_RESULTS
    import ml_dtypes
    bf16 = ml_dtypes.bfloat16
    nc, consts = _get_state()

    data = np.asarray(data, dtype=np.float32)
    wq = np.asarray(wq, dtype=np.float32)
    wk = np.asarray(wk, dtype=np.float32)
    wv = np.asarray(wv, dtype=np.float32)
    wo = np.asarray(wo, dtype=np.float32)

    in_maps = []
    # dataT host layout [128, chunk, ko, t]: every DMA element is >=1KB and
    # per-(partition, chunk) spans are 16KB contiguous
    dTs = [np.ascontiguousarray(
        data[b].T.reshape(NKO, 128, NPC, PC).transpose(1, 2, 0, 3)
    ).astype(bf16) for b in range(B)]
    wq_h = [np.ascontiguousarray(
        wq[:, g * GQ:(g + 1) * GQ].reshape(NKO, 128, GQ).transpose(1, 0, 2)
    ).astype(bf16) for g in range(NKV)]
    wkv_h = [np.ascontiguousarray(
        np.concatenate([wk[:, g * HD:(g + 1) * HD],
                        wv[:, g * HD:(g + 1) * HD]], axis=1)
        .reshape(NKO, 128, 2 * HD).transpose(1, 0, 2)
    ).astype(bf16) for g in range(NKV)]
    for b in range(B):
        for g in range(NKV):
            in_maps.append({
                "dataT": dTs[b],
                "wq": wq_h[g],
                "wkv": wkv_h[g],
                "wo": np.ascontiguousarray(wo[g * GQ:(g + 1) * GQ, :]).astype(bf16),
                "cosT": consts["cosT"],
                "sinT": consts["sinT"],
                "ctd": consts["ctd"],
                "sgn": consts["sgn"],
                "rot": consts["rot"],
                "tril": consts["tril"],
                "ones": consts["ones"],
                "ident": consts["ident"],
            })

    res = run_bass_kernel_spmd(nc, in_maps, core_ids=list(range(8)))
    LAST_RESULTS = res

    out = np.empty((B, S, D), dtype=np.float32)
    for b in range(B):
        acc = res.results[b * NKV]["outT"].astype(np.float32).copy()
        for g in range(1, NKV):
            acc += res.results[b * NKV + g]["outT"]
        out[b] = acc.T
    return out

